# revision 1
# baseline (speedup 1.0000x reference)
"""Causal self-attention (B=4, T=2048, D=1024, H=16) on 8 TRN2 NeuronCores.

Sharding: core c handles batch b = c // 2 and head-group g = c % 2
(8 heads = 512 of the 1024 feature dims). Each core:
  1. QKV projection for its head-group's columns. q, k are produced
     TRANSPOSED ([feat, tok], feature dim on partitions) so they feed the
     attention matmuls directly; v is produced natural ([tok, feat]) so it
     is the PV stationary operand.
  2. RoPE via a PE rotation matmul (rotate_half as a constant 128x128
     block-diagonal permutation) + DVE combine with cos/sin.
  3. Causal attention with scores in [k, q] orientation: exp without
     max-subtraction (scores/8 are O(1); fp32/bf16 safe), row-sum obtained
     free via a ones-column appended to v (PV matmul M=65: rows 0-63 = y,
     row 64 = softmax denominator).
  4. Late softmax normalization (reciprocal + gpsimd partition-broadcast),
     then the partial output projection with its 512 rows of W_out.
Host sums the two head-group partials per batch and adds b_out.

All matmuls run in bf16 (fp32 matmul is 1/4 rate on the PE); softmax
statistics accumulate in fp32 PSUM.
"""

import numpy as np
import ml_dtypes

import concourse.tile as tile
from concourse import bacc, mybir
from concourse.bass_utils import run_bass_kernel_spmd

dt = mybir.dt
bf16 = ml_dtypes.bfloat16

B, T, C = 4, 2048, 1024
H, HD = 16, 64
N_CORES = 8
HPC = 8          # heads per core
FPC = H // 2 * HD // 8 * 8 // 2 * 2  # = 512 features per core (q, k, v each)
KSUB = C // 128  # 8 contraction subtiles
TT = T // 128    # 16 token tiles
TC = T // 512    # 4 token chunks

_compiled = None


def _build():
    nc = bacc.Bacc()
    dts = dt.bfloat16

    xT = nc.dram_tensor("xT", [TC, 128, KSUB, 512], dts, kind="ExternalInput")
    Wqk = nc.dram_tensor("Wqk", [8, 128, KSUB, 128], dts, kind="ExternalInput")
    Wv = nc.dram_tensor("Wv", [128, KSUB, 512], dts, kind="ExternalInput")
    Wo = nc.dram_tensor("Wo", [128, 4, 1024], dts, kind="ExternalInput")
    bqk = nc.dram_tensor("bqk", [128, 8], dt.float32, kind="ExternalInput")
    bv = nc.dram_tensor("bv", [128, 512], dt.float32, kind="ExternalInput")
    RT = nc.dram_tensor("RT", [128, 128], dts, kind="ExternalInput")
    cosd = nc.dram_tensor("cosd", [TC, 128, 512], dts, kind="ExternalInput")
    sind = nc.dram_tensor("sind", [TC, 128, 512], dts, kind="ExternalInput")
    trid = nc.dram_tensor("trid", [128, 128], dts, kind="ExternalInput")
    out = nc.dram_tensor("out", [T, C], dt.float32, kind="ExternalOutput")

    with tile.TileContext(nc) as tc:
        with (
            tc.tile_pool(name="weights", bufs=1) as wp,
            tc.tile_pool(name="acts", bufs=1) as ap,
            tc.tile_pool(name="scratch", bufs=3) as sp,
            tc.tile_pool(name="exps", bufs=10) as ep,
            tc.tile_pool(name="norm", bufs=2) as np_,
            tc.tile_pool(name="outs", bufs=6) as op,
            tc.tile_pool(name="psum", bufs=2, space="PSUM") as pp,
            tc.tile_pool(name="psum_big", bufs=2, space="PSUM") as pb,
            tc.tile_pool(name="psum_pv", bufs=2, space="PSUM") as ppv,
        ):
            xT_sb = wp.tile([128, KSUB, T], dts)
            Wqk_sb = wp.tile([128, KSUB, 8, 128], dts)
            Wv_sb = wp.tile([128, KSUB, 512], dts)
            Wo_sb = wp.tile([128, 4, 1024], dts)
            bqk_sb = wp.tile([128, 8], dt.float32)
            bv_sb = wp.tile([128, 512], dt.float32)
            RT_sb = wp.tile([128, 128], dts)
            cos_sb = wp.tile([128, T], dts)
            sin_sb = wp.tile([128, T], dts)
            tri_sb = wp.tile([128, 128], dts)

            xTv = xT_sb.rearrange("p k (c q) -> p k c q", c=TC)

            def load_xT(c4):
                nc.sync.dma_start(xTv[:, :, c4, :].rearrange("p k q -> p k q"),
                                  xT[c4])

            def load_wqk(fs):
                nc.sync.dma_start(Wqk_sb[:, :, fs, :], Wqk[fs])

            # first-needed data up front, the rest behind it
            load_xT(0)
            load_wqk(0)
            load_wqk(4)
            nc.sync.dma_start(Wv_sb[:], Wv[:])
            cosv = cos_sb.rearrange("p (c q) -> p c q", c=TC)
            sinv = sin_sb.rearrange("p (c q) -> p c q", c=TC)
            nc.sync.dma_start(cosv[:, 0], cosd[0])
            nc.sync.dma_start(sinv[:, 0], sind[0])
            nc.sync.dma_start(bqk_sb[:], bqk[:])
            nc.sync.dma_start(bv_sb[:], bv[:])
            nc.sync.dma_start(RT_sb[:], RT[:])
            nc.sync.dma_start(tri_sb[:], trid[:])
            for c4 in range(1, TC):
                load_xT(c4)
                nc.sync.dma_start(cosv[:, c4], cosd[c4])
                nc.sync.dma_start(sinv[:, c4], sind[c4])
            for fs in (1, 5, 2, 6, 3, 7):
                load_wqk(fs)
            nc.sync.dma_start(Wo_sb[:], Wo[:])

            qT_sb = ap.tile([128, 4, T], dts)   # rope'd q, [feat, tok]
            kT_sb = ap.tile([128, 4, T], dts)   # rope'd k, [feat, tok]
            v_sb = ap.tile([128, TT, 8 * 65], dts)  # v natural + ones col per head
            y_sb = ap.tile([128, 4, T], dts)    # normalized attention out (lhsT)

            # ones columns of v (col 64 of each head's 65-wide block)
            v_heads = v_sb.rearrange("p t (h f) -> p t h f", h=8)
            nc.vector.memset(v_heads[:, :, :, 64], 1.0)

            # ---- fine-grained work emitters -----------------------------
            def v_tile(tt):
                psv = pp.tile([128, 512], dt.float32, tag="ps512")
                for ks in range(KSUB):
                    nc.tensor.matmul(
                        psv[:],
                        xT_sb[:, ks, tt * 128:(tt + 1) * 128],
                        Wv_sb[:, ks, :],
                        start=(ks == 0), stop=(ks == KSUB - 1),
                    )
                nc.vector.tensor_add(
                    v_heads[:, tt, :, 0:64],
                    psv[:].rearrange("p (h f) -> p h f", h=8),
                    bv_sb[:].rearrange("p (h f) -> p h f", h=8),
                )

            def proj_rope(fs, c4, use_big=False):
                tsl = slice(c4 * 512, (c4 + 1) * 512)
                if use_big:
                    # bootstrap: attention pools are idle, borrow a big tile
                    bigt = pb.tile([128, 1024], dt.float32, tag="big")
                    ps, rps = bigt[:, 0:512], bigt[:, 512:1024]
                else:
                    ps = pp.tile([128, 512], dt.float32, tag="ps512")
                    rps = pp.tile([128, 512], dt.float32, tag="ps512")
                for ks in range(KSUB):
                    nc.tensor.matmul(
                        ps[:],
                        Wqk_sb[:, ks, fs, :],
                        xT_sb[:, ks, tsl],
                        start=(ks == 0), stop=(ks == KSUB - 1),
                    )
                qb = sp.tile([128, 512], dt.float32, tag="qb")
                nc.vector.tensor_scalar_add(qb[:], ps[:], bqk_sb[:, fs:fs + 1])
                u = sp.tile([128, 512], dts, tag="u")
                nc.vector.tensor_mul(u[:], qb[:], sin_sb[:, tsl])
                w = sp.tile([128, 512], dt.float32, tag="w")
                nc.vector.tensor_mul(w[:], qb[:], cos_sb[:, tsl])
                nc.tensor.matmul(rps[:], RT_sb[:], u[:], start=True, stop=True)
                dst = qT_sb if fs < 4 else kT_sb
                nc.vector.tensor_add(dst[:, fs % 4, tsl], w[:], rps[:])

            def out_proj(tt):
                for n2 in range(2):
                    po = pp.tile([128, 512], dt.float32, tag="ps512")
                    for s in range(4):
                        nc.tensor.matmul(
                            po[:],
                            y_sb[:, s, tt * 128:(tt + 1) * 128],
                            Wo_sb[:, s, n2 * 512:(n2 + 1) * 512],
                            start=(s == 0), stop=(s == 3),
                        )
                    ost = op.tile([128, 512], dt.float32, tag="ost")
                    nc.vector.tensor_copy(ost[:], po[:])
                    nc.sync.dma_start(
                        out[tt * 128:(tt + 1) * 128, n2 * 512:(n2 + 1) * 512],
                        ost[:],
                    )

            from collections import deque

            # work items streamed into the attention pipeline's PE gaps
            fillers = deque()
            for tt in range(4, TT):
                fillers.append(("v", tt))
            for nhp in (1, 2, 3):
                for c4 in range(TC):
                    fillers.append(("proj", nhp, c4))
                    fillers.append(("proj", nhp + 4, c4))

            def run_item(it):
                if it[0] == "v":
                    v_tile(it[1])
                elif it[0] == "proj":
                    proj_rope(it[1], it[2])
                else:
                    out_proj(it[1])

            def pop_filler():
                if fillers:
                    run_item(fillers.popleft())

            def drain_while(pred):
                while fillers and pred(fillers[0]):
                    run_item(fillers.popleft())

            # bootstrap: head pair 0's q/k for all chunks + v for chunk 0
            for c4 in range(TC):
                proj_rope(0, c4, use_big=True)
                proj_rope(4, c4, use_big=True)
            for tt in range(4):
                v_tile(tt)

            for hp in range(4):
                for qc in range(TC):
                    # dependencies of this attention block must be emitted
                    if hp == 0:
                        drain_while(lambda it: it[0] == "v" and it[1] < 4 * qc + 4)
                    else:
                        drain_while(lambda it: it[0] == "v" or (
                            it[0] == "proj" and it[1] % 4 == hp))
                    qsl = slice(qc * 512, (qc + 1) * 512)
                    jmax = 4 * qc + 3
                    pv0 = ppv.tile([65, 512], dt.float32, tag="pv")
                    pv1 = ppv.tile([65, 512], dt.float32, tag="pv")
                    for j in range(jmax + 1):
                        qs = max(0, j * 128 - qc * 512)
                        w = 512 - qs
                        # both parities' scores side by side in one 2-bank tile
                        big = pb.tile([128, 1024], dt.float32, tag="big")
                        for par in range(2):
                            kb = par * 64
                            nc.tensor.matmul(
                                big[:, par * 512 + qs:par * 512 + 512],
                                kT_sb[kb:kb + 64, hp, j * 128:(j + 1) * 128],
                                qT_sb[kb:kb + 64, hp, qc * 512 + qs:(qc + 1) * 512],
                                start=True, stop=True,
                            )
                        ex = ep.tile([128, 1024], dts, tag="ex")
                        big_v = big.rearrange("p (two q) -> p two q", two=2)
                        ex_v = ex.rearrange("p (two q) -> p two q", two=2)
                        nc.scalar.activation(
                            ex_v[:, :, qs:512], big_v[:, :, qs:512],
                            mybir.ActivationFunctionType.Exp,
                            bias=0.0, scale=0.125,
                        )
                        if qs > 0 or j * 128 == qc * 512:
                            # diagonal tile: zero the strictly-upper part
                            nc.vector.tensor_tensor(
                                ex_v[:, :, qs:qs + 128],
                                ex_v[:, :, qs:qs + 128],
                                tri_sb[:, None, :].to_broadcast((128, 2, 128)),
                                mybir.AluOpType.mult,
                            )
                        for par in range(2):
                            h = 2 * hp + par
                            pv = pv0 if par == 0 else pv1
                            nc.tensor.matmul(
                                pv[:, qs:512],
                                v_sb[:, j, h * 65:(h + 1) * 65],
                                ex[:, par * 512 + qs:par * 512 + 512],
                                start=(j == 0), stop=(j == jmax),
                            )
                        pop_filler()
                    for par in range(2):
                        h = 2 * hp + par
                        kb = par * 64
                        pv = pv0 if par == 0 else pv1
                        rinv = np_.tile([1, 512], dt.float32, tag="rinv")
                        nc.vector.reciprocal(rinv[0:1, :], pv[64:65, :])
                        rb = np_.tile([64, 512], dt.float32, tag="rb")
                        nc.gpsimd.partition_broadcast(rb[:], rinv[0:1, :])
                        nc.vector.tensor_mul(
                            y_sb[kb:kb + 64, hp, qsl], pv[0:64, :], rb[:],
                        )
                    if hp == 3:
                        # this chunk's y is final for all heads: stream out-proj
                        for tt in range(4 * qc, 4 * qc + 4):
                            fillers.append(("out", tt))
            while fillers:
                run_item(fillers.popleft())

    nc.compile()
    return nc


def _prep_core_inputs(x, W_qkv, b_qkv, W_out, g):
    """Host-side shard prep for head-group g (features g*512:(g+1)*512)."""
    fs = slice(g * 512, (g + 1) * 512)
    Wq = W_qkv[:, 0:1024][:, fs]          # [1024, 512]
    Wk = W_qkv[:, 1024:2048][:, fs]
    Wv_ = W_qkv[:, 2048:3072][:, fs]
    bq = b_qkv[0:1024][fs]
    bk = b_qkv[1024:2048][fs]
    bv_ = b_qkv[2048:3072][fs]

    Wqk_np = np.concatenate([Wq, Wk], axis=1)        # [1024, 1024]
    # [fs, p, ks, col]
    Wqk_np = Wqk_np.reshape(KSUB, 128, 8, 128).transpose(2, 1, 0, 3)
    Wv_np = Wv_.reshape(KSUB, 128, 512).transpose(1, 0, 2)
    Wo_np = W_out[fs, :].reshape(4, 128, 1024).transpose(1, 0, 2)
    bqk_np = np.concatenate([bq, bk]).reshape(8, 128).T.copy()   # [128, 8]
    bv_np = np.broadcast_to(bv_[None, :], (128, 512)).copy()

    return {
        "Wqk": np.ascontiguousarray(Wqk_np).astype(bf16),
        "Wv": np.ascontiguousarray(Wv_np).astype(bf16),
        "Wo": np.ascontiguousarray(Wo_np).astype(bf16),
        "bqk": np.ascontiguousarray(bqk_np).astype(np.float32),
        "bv": bv_np.astype(np.float32),
    }


def _shared_inputs():
    # rotation matrix: (R q)[d] = -q[d+32] for d<32, q[d-32] for 32<=d<64
    R64 = np.zeros((64, 64), dtype=np.float32)
    for d in range(32):
        R64[d, d + 32] = -1.0
        R64[d + 32, d] = 1.0
    R128 = np.zeros((128, 128), dtype=np.float32)
    R128[0:64, 0:64] = R64
    R128[64:128, 64:128] = R64
    RT_np = R128.T.copy()

    inv_freq = 1.0 / (10000.0 ** (np.arange(0, HD, 2, dtype=np.float32) / HD))
    t = np.arange(T, dtype=np.float32)
    freqs = np.outer(t, inv_freq)                     # [T, 32]
    p = np.arange(128)
    cos_np = np.cos(freqs[:, p % 32]).T.copy()        # [128, T]
    sin_np = np.sin(freqs[:, p % 32]).T.copy()

    tri_np = np.tril(np.ones((128, 128), dtype=np.float32)).T.copy()  # kk<=qq

    return {
        "RT": RT_np.astype(bf16),
        "cosd": np.ascontiguousarray(
            cos_np.reshape(128, TC, 512).transpose(1, 0, 2)).astype(bf16),
        "sind": np.ascontiguousarray(
            sin_np.reshape(128, TC, 512).transpose(1, 0, 2)).astype(bf16),
        "trid": np.ascontiguousarray(tri_np).astype(bf16),
    }


def run(x, W_qkv, b_qkv, W_out, b_out, trace=False):
    global _compiled
    if _compiled is None:
        _compiled = _build()
    nc = _compiled

    shared = _shared_inputs()
    group_inp = [_prep_core_inputs(x, W_qkv, b_qkv, W_out, g) for g in range(2)]

    in_maps = []
    for core in range(N_CORES):
        b, g = core // 2, core % 2
        # [c4, p, ks, q]
        xT_np = (x[b].reshape(TC, 512, KSUB, 128).transpose(0, 3, 2, 1))
        m = {"xT": np.ascontiguousarray(xT_np).astype(bf16)}
        m.update(group_inp[g])
        m.update(shared)
        in_maps.append(m)

    res = run_bass_kernel_spmd(
        nc, in_maps, core_ids=list(range(N_CORES)), trace=trace,
        stitch_traces=trace,
    )
    outp = np.empty((B, T, C), dtype=np.float32)
    for b in range(B):
        outp[b] = (res.results[2 * b]["out"] + res.results[2 * b + 1]["out"]
                   + b_out[None, :])
    return outp, res


def kernel(x, W_qkv, b_qkv, W_out, b_out):
    x = np.asarray(x, dtype=np.float32)
    W_qkv = np.asarray(W_qkv, dtype=np.float32)
    b_qkv = np.asarray(b_qkv, dtype=np.float32)
    W_out = np.asarray(W_out, dtype=np.float32)
    b_out = np.asarray(b_out, dtype=np.float32)
    outp, _ = run(x, W_qkv, b_qkv, W_out, b_out, trace=False)
    return outp



# revision 5
# speedup vs baseline: 1.0118x; 1.0118x over previous
"""Causal self-attention (B=4, T=2048, D=1024, H=16) on 8 TRN2 NeuronCores.

Sharding: core c handles batch b = c // 2 and head-group g = c % 2
(8 heads = 512 of the 1024 feature dims). Each core:
  1. QKV projection for its head-group's columns. q, k are produced
     TRANSPOSED ([feat, tok], feature dim on partitions) so they feed the
     attention matmuls directly; v is produced natural ([tok, feat]) so it
     is the PV stationary operand.
  2. RoPE via a PE rotation matmul (rotate_half as a constant 128x128
     block-diagonal permutation) + DVE combine with cos/sin.
  3. Causal attention with scores in [k, q] orientation: exp without
     max-subtraction (scores/8 are O(1); fp32/bf16 safe), row-sum obtained
     free via a ones-column appended to v (PV matmul M=65: rows 0-63 = y,
     row 64 = softmax denominator).
  4. Late softmax normalization (reciprocal + gpsimd partition-broadcast),
     then the partial output projection with its 512 rows of W_out.
Host sums the two head-group partials per batch and adds b_out.

All matmuls run in bf16 (fp32 matmul is 1/4 rate on the PE); softmax
statistics accumulate in fp32 PSUM.
"""

import numpy as np
import ml_dtypes

import concourse.tile as tile
from concourse import bacc, mybir
from concourse.bass_utils import run_bass_kernel_spmd

dt = mybir.dt
bf16 = ml_dtypes.bfloat16

B, T, C = 4, 2048, 1024
H, HD = 16, 64
N_CORES = 8
HPC = 8          # heads per core
FPC = H // 2 * HD // 8 * 8 // 2 * 2  # = 512 features per core (q, k, v each)
KSUB = C // 128  # 8 contraction subtiles
TT = T // 128    # 16 token tiles
TC = T // 512    # 4 token chunks

_compiled = None


def _build():
    nc = bacc.Bacc()
    dts = dt.bfloat16

    xT = nc.dram_tensor("xT", [TC, 128, KSUB, 512], dts, kind="ExternalInput")
    Wqk = nc.dram_tensor("Wqk", [8, 128, KSUB, 128], dts, kind="ExternalInput")
    Wv = nc.dram_tensor("Wv", [128, KSUB, 512], dts, kind="ExternalInput")
    Wo = nc.dram_tensor("Wo", [128, 4, 1024], dts, kind="ExternalInput")
    bqk = nc.dram_tensor("bqk", [128, 8], dt.float32, kind="ExternalInput")
    bv = nc.dram_tensor("bv", [128, 512], dt.float32, kind="ExternalInput")
    RT = nc.dram_tensor("RT", [128, 128], dts, kind="ExternalInput")
    cosd = nc.dram_tensor("cosd", [TC, 128, 512], dts, kind="ExternalInput")
    sind = nc.dram_tensor("sind", [TC, 128, 512], dts, kind="ExternalInput")
    trid = nc.dram_tensor("trid", [128, 128], dts, kind="ExternalInput")
    out = nc.dram_tensor("out", [T, C], dt.float32, kind="ExternalOutput")

    with tile.TileContext(nc) as tc:
        with (
            tc.tile_pool(name="weights", bufs=1) as wp,
            tc.tile_pool(name="acts", bufs=1) as ap,
            tc.tile_pool(name="scratch", bufs=3) as sp,
            tc.tile_pool(name="exps", bufs=10) as ep,
            tc.tile_pool(name="norm", bufs=2) as np_,
            tc.tile_pool(name="outs", bufs=6) as op,
            tc.tile_pool(name="psum", bufs=2, space="PSUM") as pp,
            tc.tile_pool(name="psum_big", bufs=2, space="PSUM") as pb,
            tc.tile_pool(name="psum_pv", bufs=2, space="PSUM") as ppv,
        ):
            xT_sb = wp.tile([128, KSUB, T], dts)
            Wqk_sb = wp.tile([128, KSUB, 8, 128], dts)
            Wv_sb = wp.tile([128, KSUB, 512], dts)
            Wo_sb = wp.tile([128, 4, 1024], dts)
            bqk_sb = wp.tile([128, 8], dt.float32)
            bv_sb = wp.tile([128, 512], dt.float32)
            RT_sb = wp.tile([128, 128], dts)
            cos_sb = wp.tile([128, T], dts)
            sin_sb = wp.tile([128, T], dts)
            tri_sb = wp.tile([128, 128], dts)

            xTv = xT_sb.rearrange("p k (c q) -> p k c q", c=TC)

            def load_xT(c4):
                nc.sync.dma_start(xTv[:, :, c4, :].rearrange("p k q -> p k q"),
                                  xT[c4])

            def load_wqk(fs):
                nc.sync.dma_start(Wqk_sb[:, :, fs, :], Wqk[fs])

            # first-needed data up front in fine grain so the first proj
            # matmuls can start ~1us in, the rest behind it
            cosv = cos_sb.rearrange("p (c q) -> p c q", c=TC)
            sinv = sin_sb.rearrange("p (c q) -> p c q", c=TC)
            load_wqk(0)
            nc.sync.dma_start(bqk_sb[:], bqk[:])
            nc.sync.dma_start(RT_sb[:], RT[:])
            nc.sync.dma_start(cosv[:, 0], cosd[0])
            nc.sync.dma_start(sinv[:, 0], sind[0])
            for ks2 in range(0, KSUB, 2):  # chunk-0 tokens, 2 k-subtiles apiece
                nc.sync.dma_start(xTv[:, ks2:ks2 + 2, 0, :], xT[0, :, ks2:ks2 + 2, :])
            load_wqk(4)
            nc.sync.dma_start(Wv_sb[:], Wv[:])
            nc.sync.dma_start(bv_sb[:], bv[:])
            nc.sync.dma_start(tri_sb[:], trid[:])
            for c4 in range(1, TC):
                load_xT(c4)
                nc.sync.dma_start(cosv[:, c4], cosd[c4])
                nc.sync.dma_start(sinv[:, c4], sind[c4])
            for fs in (1, 5, 2, 6, 3, 7):
                load_wqk(fs)
            nc.sync.dma_start(Wo_sb[:], Wo[:])

            qT_sb = ap.tile([128, 4, T], dts)   # rope'd q, [feat, tok]
            kT_sb = ap.tile([128, 4, T], dts)   # rope'd k, [feat, tok]
            v_sb = ap.tile([128, TT, 8 * 65], dts)  # v natural + ones col per head
            y_sb = ap.tile([128, 4, T], dts)    # normalized attention out (lhsT)

            # ones columns of v (col 64 of each head's 65-wide block)
            v_heads = v_sb.rearrange("p t (h f) -> p t h f", h=8)
            nc.vector.memset(v_heads[:, :, :, 64], 1.0)

            # ---- fine-grained work emitters -----------------------------
            def v_tile(tt):
                psv = pp.tile([128, 512], dt.float32, tag="ps512")
                for ks in range(KSUB):
                    nc.tensor.matmul(
                        psv[:],
                        xT_sb[:, ks, tt * 128:(tt + 1) * 128],
                        Wv_sb[:, ks, :],
                        start=(ks == 0), stop=(ks == KSUB - 1),
                    )
                nc.vector.tensor_add(
                    v_heads[:, tt, :, 0:64],
                    psv[:].rearrange("p (h f) -> p h f", h=8),
                    bv_sb[:].rearrange("p (h f) -> p h f", h=8),
                )

            def proj_rope(fs, c4, use_big=False):
                tsl = slice(c4 * 512, (c4 + 1) * 512)
                if use_big:
                    # bootstrap: attention pools are idle, borrow a big tile
                    bigt = pb.tile([128, 1024], dt.float32, tag="big")
                    ps, rps = bigt[:, 0:512], bigt[:, 512:1024]
                else:
                    ps = pp.tile([128, 512], dt.float32, tag="ps512")
                    rps = pp.tile([128, 512], dt.float32, tag="ps512")
                for ks in range(KSUB):
                    nc.tensor.matmul(
                        ps[:],
                        Wqk_sb[:, ks, fs, :],
                        xT_sb[:, ks, tsl],
                        start=(ks == 0), stop=(ks == KSUB - 1),
                    )
                qb = sp.tile([128, 512], dt.float32, tag="qb")
                nc.vector.tensor_scalar_add(qb[:], ps[:], bqk_sb[:, fs:fs + 1])
                u = sp.tile([128, 512], dts, tag="u")
                nc.vector.tensor_mul(u[:], qb[:], sin_sb[:, tsl])
                w = sp.tile([128, 512], dt.float32, tag="w")
                nc.vector.tensor_mul(w[:], qb[:], cos_sb[:, tsl])
                nc.tensor.matmul(rps[:], RT_sb[:], u[:], start=True, stop=True)
                dst = qT_sb if fs < 4 else kT_sb
                nc.vector.tensor_add(dst[:, fs % 4, tsl], w[:], rps[:])

            def out_proj(tt):
                for n2 in range(2):
                    po = pp.tile([128, 512], dt.float32, tag="ps512")
                    for s in range(4):
                        nc.tensor.matmul(
                            po[:],
                            y_sb[:, s, tt * 128:(tt + 1) * 128],
                            Wo_sb[:, s, n2 * 512:(n2 + 1) * 512],
                            start=(s == 0), stop=(s == 3),
                        )
                    ost = op.tile([128, 512], dt.float32, tag="ost")
                    nc.vector.tensor_copy(ost[:], po[:])
                    nc.sync.dma_start(
                        out[tt * 128:(tt + 1) * 128, n2 * 512:(n2 + 1) * 512],
                        ost[:],
                    )

            from collections import deque

            # Filler work items (qkv projections, v tiles, out projections)
            # streamed into the attention pipeline's PE gaps.  The attention
            # inner loop is Activation-paced (exp ~1040ns vs 4 matmuls
            # ~850ns per k-tile step), so the PE has a ~300-500ns deficit
            # per step that filler matmuls must cover; pacing them evenly
            # across ALL 160 steps (instead of draining greedily up front)
            # is what keeps the PE busy end to end.
            # Item = (deadline_block, kind, *args); FIFO is deadline-sorted.
            PE_MM = 213.0  # ns per 512-row bf16 matmul at full clock
            ITEM_COST = {"v": 8 * PE_MM, "proj": 9 * PE_MM, "out": 8 * PE_MM}
            DEFICIT_PER_J = 450.0
            BLOCK_BONUS = 600.0

            fillers = deque()
            for c4 in range(1, TC):  # head-pair 0's remaining chunks + v
                fillers.append((c4, "proj", 0, c4))
                fillers.append((c4, "proj", 4, c4))
                for tt in range(4 * c4, 4 * c4 + 4):
                    fillers.append((c4, "v", tt))
            for hp in range(1, 4):
                for c4 in range(TC):
                    fillers.append((hp * 4 + c4, "proj", hp, c4))
                    fillers.append((hp * 4 + c4, "proj", hp + 4, c4))

            deficit = 0.0

            def run_item(it):
                if it[1] == "v":
                    v_tile(it[2])
                elif it[1] == "proj":
                    proj_rope(it[2], it[3])
                else:
                    out_proj(it[2])

            def pop_by_deficit():
                nonlocal deficit
                while fillers and deficit >= ITEM_COST[fillers[0][1]]:
                    it = fillers.popleft()
                    deficit -= ITEM_COST[it[1]]
                    run_item(it)

            # bootstrap: only block (0,0)'s dependencies
            proj_rope(0, 0, use_big=True)
            proj_rope(4, 0, use_big=True)
            for tt in range(4):
                v_tile(tt)

            for hp in range(4):
                for qc in range(TC):
                    bidx = hp * 4 + qc
                    # dependencies of this attention block must be emitted
                    while fillers and fillers[0][0] <= bidx:
                        run_item(fillers.popleft())
                    qsl = slice(qc * 512, (qc + 1) * 512)
                    jmax = 4 * qc + 3
                    pv0 = ppv.tile([65, 512], dt.float32, tag="pv")
                    pv1 = ppv.tile([65, 512], dt.float32, tag="pv")
                    for j in range(jmax + 1):
                        qs = max(0, j * 128 - qc * 512)
                        w = 512 - qs
                        # both parities' scores side by side in one 2-bank tile
                        big = pb.tile([128, 1024], dt.float32, tag="big")
                        for par in range(2):
                            kb = par * 64
                            nc.tensor.matmul(
                                big[:, par * 512 + qs:par * 512 + 512],
                                kT_sb[kb:kb + 64, hp, j * 128:(j + 1) * 128],
                                qT_sb[kb:kb + 64, hp, qc * 512 + qs:(qc + 1) * 512],
                                start=True, stop=True,
                            )
                        ex = ep.tile([128, 1024], dts, tag="ex")
                        big_v = big.rearrange("p (two q) -> p two q", two=2)
                        ex_v = ex.rearrange("p (two q) -> p two q", two=2)
                        nc.scalar.activation(
                            ex_v[:, :, qs:512], big_v[:, :, qs:512],
                            mybir.ActivationFunctionType.Exp,
                            bias=0.0, scale=0.125,
                        )
                        if qs > 0 or j * 128 == qc * 512:
                            # diagonal tile: zero the strictly-upper part
                            nc.vector.tensor_tensor(
                                ex_v[:, :, qs:qs + 128],
                                ex_v[:, :, qs:qs + 128],
                                tri_sb[:, None, :].to_broadcast((128, 2, 128)),
                                mybir.AluOpType.mult,
                            )
                        for par in range(2):
                            h = 2 * hp + par
                            pv = pv0 if par == 0 else pv1
                            nc.tensor.matmul(
                                pv[:, qs:512],
                                v_sb[:, j, h * 65:(h + 1) * 65],
                                ex[:, par * 512 + qs:par * 512 + 512],
                                start=(j == 0), stop=(j == jmax),
                            )
                        deficit += DEFICIT_PER_J
                        pop_by_deficit()
                    for par in range(2):
                        h = 2 * hp + par
                        kb = par * 64
                        pv = pv0 if par == 0 else pv1
                        rinv = np_.tile([1, 512], dt.float32, tag="rinv")
                        nc.vector.reciprocal(rinv[0:1, :], pv[64:65, :])
                        rb = np_.tile([64, 512], dt.float32, tag="rb")
                        nc.gpsimd.partition_broadcast(rb[:], rinv[0:1, :])
                        nc.vector.tensor_mul(
                            y_sb[kb:kb + 64, hp, qsl], pv[0:64, :], rb[:],
                        )
                    deficit += BLOCK_BONUS
                    if hp == 3:
                        # this chunk's y is final for all heads: stream out-proj
                        for tt in range(4 * qc, 4 * qc + 4):
                            fillers.append((16, "out", tt))
            while fillers:
                run_item(fillers.popleft())

    nc.compile()
    return nc


def _prep_core_inputs(x, W_qkv, b_qkv, W_out, g):
    """Host-side shard prep for head-group g (features g*512:(g+1)*512)."""
    fs = slice(g * 512, (g + 1) * 512)
    Wq = W_qkv[:, 0:1024][:, fs]          # [1024, 512]
    Wk = W_qkv[:, 1024:2048][:, fs]
    Wv_ = W_qkv[:, 2048:3072][:, fs]
    bq = b_qkv[0:1024][fs]
    bk = b_qkv[1024:2048][fs]
    bv_ = b_qkv[2048:3072][fs]

    Wqk_np = np.concatenate([Wq, Wk], axis=1)        # [1024, 1024]
    # [fs, p, ks, col]
    Wqk_np = Wqk_np.reshape(KSUB, 128, 8, 128).transpose(2, 1, 0, 3)
    Wv_np = Wv_.reshape(KSUB, 128, 512).transpose(1, 0, 2)
    Wo_np = W_out[fs, :].reshape(4, 128, 1024).transpose(1, 0, 2)
    bqk_np = np.concatenate([bq, bk]).reshape(8, 128).T.copy()   # [128, 8]
    bv_np = np.broadcast_to(bv_[None, :], (128, 512)).copy()

    return {
        "Wqk": np.ascontiguousarray(Wqk_np).astype(bf16),
        "Wv": np.ascontiguousarray(Wv_np).astype(bf16),
        "Wo": np.ascontiguousarray(Wo_np).astype(bf16),
        "bqk": np.ascontiguousarray(bqk_np).astype(np.float32),
        "bv": bv_np.astype(np.float32),
    }


def _shared_inputs():
    # rotation matrix: (R q)[d] = -q[d+32] for d<32, q[d-32] for 32<=d<64
    R64 = np.zeros((64, 64), dtype=np.float32)
    for d in range(32):
        R64[d, d + 32] = -1.0
        R64[d + 32, d] = 1.0
    R128 = np.zeros((128, 128), dtype=np.float32)
    R128[0:64, 0:64] = R64
    R128[64:128, 64:128] = R64
    RT_np = R128.T.copy()

    inv_freq = 1.0 / (10000.0 ** (np.arange(0, HD, 2, dtype=np.float32) / HD))
    t = np.arange(T, dtype=np.float32)
    freqs = np.outer(t, inv_freq)                     # [T, 32]
    p = np.arange(128)
    cos_np = np.cos(freqs[:, p % 32]).T.copy()        # [128, T]
    sin_np = np.sin(freqs[:, p % 32]).T.copy()

    tri_np = np.tril(np.ones((128, 128), dtype=np.float32)).T.copy()  # kk<=qq

    return {
        "RT": RT_np.astype(bf16),
        "cosd": np.ascontiguousarray(
            cos_np.reshape(128, TC, 512).transpose(1, 0, 2)).astype(bf16),
        "sind": np.ascontiguousarray(
            sin_np.reshape(128, TC, 512).transpose(1, 0, 2)).astype(bf16),
        "trid": np.ascontiguousarray(tri_np).astype(bf16),
    }


def run(x, W_qkv, b_qkv, W_out, b_out, trace=False):
    global _compiled
    if _compiled is None:
        _compiled = _build()
    nc = _compiled

    shared = _shared_inputs()
    group_inp = [_prep_core_inputs(x, W_qkv, b_qkv, W_out, g) for g in range(2)]

    in_maps = []
    for core in range(N_CORES):
        b, g = core // 2, core % 2
        # [c4, p, ks, q]
        xT_np = (x[b].reshape(TC, 512, KSUB, 128).transpose(0, 3, 2, 1))
        m = {"xT": np.ascontiguousarray(xT_np).astype(bf16)}
        m.update(group_inp[g])
        m.update(shared)
        in_maps.append(m)

    res = run_bass_kernel_spmd(
        nc, in_maps, core_ids=list(range(N_CORES)), trace=trace,
        stitch_traces=trace,
    )
    outp = np.empty((B, T, C), dtype=np.float32)
    for b in range(B):
        outp[b] = (res.results[2 * b]["out"] + res.results[2 * b + 1]["out"]
                   + b_out[None, :])
    return outp, res


def kernel(x, W_qkv, b_qkv, W_out, b_out):
    x = np.asarray(x, dtype=np.float32)
    W_qkv = np.asarray(W_qkv, dtype=np.float32)
    b_qkv = np.asarray(b_qkv, dtype=np.float32)
    W_out = np.asarray(W_out, dtype=np.float32)
    b_out = np.asarray(b_out, dtype=np.float32)
    outp, _ = run(x, W_qkv, b_qkv, W_out, b_out, trace=False)
    return outp



# revision 9
# speedup vs baseline: 1.0186x; 1.0067x over previous
"""Causal self-attention (B=4, T=2048, D=1024, H=16) on 8 TRN2 NeuronCores.

Sharding: core c handles batch b = c // 2 and head-group g = c % 2
(8 heads = 512 of the 1024 feature dims). Each core:
  1. QKV projection for its head-group's columns. q, k are produced
     TRANSPOSED ([feat, tok], feature dim on partitions) so they feed the
     attention matmuls directly; v is produced natural ([tok, feat]) so it
     is the PV stationary operand.
  2. RoPE via a PE rotation matmul (rotate_half as a constant 128x128
     block-diagonal permutation) + DVE combine with cos/sin.
  3. Causal attention with scores in [k, q] orientation: exp without
     max-subtraction (scores/8 are O(1); fp32/bf16 safe), row-sum obtained
     free via a ones-column appended to v (PV matmul M=65: rows 0-63 = y,
     row 64 = softmax denominator).
  4. Late softmax normalization (reciprocal + gpsimd partition-broadcast),
     then the partial output projection with its 512 rows of W_out.
Host sums the two head-group partials per batch and adds b_out.

All matmuls run in bf16 (fp32 matmul is 1/4 rate on the PE); softmax
statistics accumulate in fp32 PSUM.
"""

import numpy as np
import ml_dtypes

import concourse.tile as tile
from concourse import bacc, mybir
from concourse.bass_utils import run_bass_kernel_spmd

dt = mybir.dt
bf16 = ml_dtypes.bfloat16

B, T, C = 4, 2048, 1024
H, HD = 16, 64
N_CORES = 8
HPC = 8          # heads per core
FPC = H // 2 * HD // 8 * 8 // 2 * 2  # = 512 features per core (q, k, v each)
KSUB = C // 128  # 8 contraction subtiles
TT = T // 128    # 16 token tiles
TC = T // 512    # 4 token chunks

_compiled = None


def _build():
    nc = bacc.Bacc()
    dts = dt.bfloat16

    xT = nc.dram_tensor("xT", [TC, 128, KSUB, 512], dts, kind="ExternalInput")
    Wqk = nc.dram_tensor("Wqk", [8, 128, KSUB, 128], dts, kind="ExternalInput")
    Wv = nc.dram_tensor("Wv", [128, KSUB, 512], dts, kind="ExternalInput")
    Wo = nc.dram_tensor("Wo", [128, 4, 1024], dts, kind="ExternalInput")
    bqk = nc.dram_tensor("bqk", [128, 8], dt.float32, kind="ExternalInput")
    bv = nc.dram_tensor("bv", [128, 512], dt.float32, kind="ExternalInput")
    RT = nc.dram_tensor("RT", [128, 128], dts, kind="ExternalInput")
    cosd = nc.dram_tensor("cosd", [TC, 128, 512], dts, kind="ExternalInput")
    sind = nc.dram_tensor("sind", [TC, 128, 512], dts, kind="ExternalInput")
    trid = nc.dram_tensor("trid", [128, 128], dts, kind="ExternalInput")
    out = nc.dram_tensor("out", [T, C], dt.float32, kind="ExternalOutput")

    with tile.TileContext(nc) as tc:
        with (
            tc.tile_pool(name="weights", bufs=1) as wp,
            tc.tile_pool(name="acts", bufs=1) as ap,
            tc.tile_pool(name="scratch", bufs=3) as sp,
            tc.tile_pool(name="exps", bufs=10) as ep,
            tc.tile_pool(name="norm", bufs=2) as np_,
            tc.tile_pool(name="outs", bufs=6) as op,
            tc.tile_pool(name="psum", bufs=2, space="PSUM") as pp,
            tc.tile_pool(name="psum_big", bufs=2, space="PSUM") as pb,
            tc.tile_pool(name="psum_pv", bufs=2, space="PSUM") as ppv,
        ):
            xT_sb = wp.tile([128, KSUB, T], dts)
            Wqk_sb = wp.tile([128, 8, KSUB, 128], dts)
            Wv_sb = wp.tile([128, KSUB, 512], dts)
            Wo_sb = wp.tile([128, 4, 1024], dts)
            bqk_sb = wp.tile([128, 8], dt.float32)
            bv_sb = wp.tile([128, 512], dt.float32)
            RT_sb = wp.tile([128, 128], dts)
            cos_sb = wp.tile([128, T], dts)
            sin_sb = wp.tile([128, T], dts)
            tri_sb = wp.tile([128, 128], dts)

            xTv = xT_sb.rearrange("p k (c q) -> p k c q", c=TC)

            def load_xT(c4):
                nc.sync.dma_start(xTv[:, :, c4, :].rearrange("p k q -> p k q"),
                                  xT[c4])

            def load_wqk(fs):
                nc.sync.dma_start(Wqk_sb[:, fs], Wqk[fs])

            # first-needed data up front in fine grain so the first proj
            # matmuls can start ~2us in, the rest behind it (all DMAs share
            # one serialized engine pool, so order = priority)
            cosv = cos_sb.rearrange("p (c q) -> p c q", c=TC)
            sinv = sin_sb.rearrange("p (c q) -> p c q", c=TC)
            load_wqk(0)
            for ks2 in range(0, KSUB, 2):  # chunk-0 tokens, 2 k-subtiles apiece
                nc.sync.dma_start(xTv[:, ks2:ks2 + 2, 0, :], xT[0, :, ks2:ks2 + 2, :])
            load_wqk(4)
            nc.sync.dma_start(bqk_sb[:], bqk[:])
            nc.sync.dma_start(RT_sb[:], RT[:])
            nc.sync.dma_start(cosv[:, 0], cosd[0])
            nc.sync.dma_start(sinv[:, 0], sind[0])
            nc.sync.dma_start(Wv_sb[:], Wv[:])
            nc.sync.dma_start(bv_sb[:], bv[:])
            nc.sync.dma_start(tri_sb[:], trid[:])
            for c4 in range(1, TC):
                load_xT(c4)
                nc.sync.dma_start(cosv[:, c4], cosd[c4])
                nc.sync.dma_start(sinv[:, c4], sind[c4])
            for fs in (1, 5, 2, 6, 3, 7):
                load_wqk(fs)
            nc.sync.dma_start(Wo_sb[:], Wo[:])

            qT_sb = ap.tile([128, 4, T], dts)   # rope'd q, [feat, tok]
            kT_sb = ap.tile([128, 4, T], dts)   # rope'd k, [feat, tok]
            v_sb = ap.tile([128, TT, 8 * 65], dts)  # v natural + ones col per head
            y_sb = ap.tile([128, 4, T], dts)    # normalized attention out (lhsT)

            # ones columns of v (col 64 of each head's 65-wide block)
            v_heads = v_sb.rearrange("p t (h f) -> p t h f", h=8)
            nc.vector.memset(v_heads[:, :, :, 64], 1.0)

            # ---- fine-grained work emitters -----------------------------
            def v_tile(tt):
                psv = pp.tile([128, 512], dt.float32, tag="ps512")
                for ks in range(KSUB):
                    nc.tensor.matmul(
                        psv[:],
                        xT_sb[:, ks, tt * 128:(tt + 1) * 128],
                        Wv_sb[:, ks, :],
                        start=(ks == 0), stop=(ks == KSUB - 1),
                    )
                nc.vector.tensor_add(
                    v_heads[:, tt, :, 0:64],
                    psv[:].rearrange("p (h f) -> p h f", h=8),
                    bv_sb[:].rearrange("p (h f) -> p h f", h=8),
                )

            def proj_rope(fs, c4, use_big=False):
                tsl = slice(c4 * 512, (c4 + 1) * 512)
                if use_big:
                    # bootstrap: attention pools are idle, borrow a big tile
                    bigt = pb.tile([128, 1024], dt.float32, tag="big")
                    ps, rps = bigt[:, 0:512], bigt[:, 512:1024]
                else:
                    ps = pp.tile([128, 512], dt.float32, tag="ps512")
                    rps = pp.tile([128, 512], dt.float32, tag="ps512")
                for ks in range(KSUB):
                    nc.tensor.matmul(
                        ps[:],
                        Wqk_sb[:, fs, ks, :],
                        xT_sb[:, ks, tsl],
                        start=(ks == 0), stop=(ks == KSUB - 1),
                    )
                qb = sp.tile([128, 512], dt.float32, tag="qb")
                nc.vector.tensor_scalar_add(qb[:], ps[:], bqk_sb[:, fs:fs + 1])
                u = sp.tile([128, 512], dts, tag="u")
                nc.vector.tensor_mul(u[:], qb[:], sin_sb[:, tsl])
                w = sp.tile([128, 512], dt.float32, tag="w")
                nc.vector.tensor_mul(w[:], qb[:], cos_sb[:, tsl])
                nc.tensor.matmul(rps[:], RT_sb[:], u[:], start=True, stop=True)
                dst = qT_sb if fs < 4 else kT_sb
                nc.vector.tensor_add(dst[:, fs % 4, tsl], w[:], rps[:])

            def out_proj(tt):
                for n2 in range(2):
                    po = pp.tile([128, 512], dt.float32, tag="ps512")
                    for s in range(4):
                        nc.tensor.matmul(
                            po[:],
                            y_sb[:, s, tt * 128:(tt + 1) * 128],
                            Wo_sb[:, s, n2 * 512:(n2 + 1) * 512],
                            start=(s == 0), stop=(s == 3),
                        )
                    ost = op.tile([128, 512], dt.float32, tag="ost")
                    nc.vector.tensor_copy(ost[:], po[:])
                    nc.sync.dma_start(
                        out[tt * 128:(tt + 1) * 128, n2 * 512:(n2 + 1) * 512],
                        ost[:],
                    )

            from collections import deque

            # Filler work items (qkv projections, v tiles, out projections)
            # streamed into the attention pipeline's PE gaps.  The attention
            # inner loop is Activation-paced (exp ~1040ns vs 4 matmuls
            # ~850ns per k-tile step), so the PE has a ~300-500ns deficit
            # per step that filler matmuls must cover; pacing them evenly
            # across ALL 160 steps (instead of draining greedily up front)
            # is what keeps the PE busy end to end.
            # Item = (deadline_block, kind, *args); FIFO is deadline-sorted.
            PE_MM = 213.0  # ns per 512-row bf16 matmul at full clock
            ITEM_COST = {"v": 8 * PE_MM, "proj": 9 * PE_MM, "out": 8 * PE_MM}
            DEFICIT_PER_J = 450.0
            BLOCK_BONUS = 600.0

            # proj items get a one-block-earlier deadline: the block's first
            # scores read qT/kT through the proj's DVE rope chain (~2.3us
            # after its last matmul), so draining at the consuming block
            # stalls the PE on DVE.  v items feed pv directly (short dep) and
            # can drain just in time.
            items = []
            for c4 in range(1, TC):  # head-pair 0's remaining chunks + v
                items.append((c4 - 1, "proj", 0, c4))
                items.append((c4 - 1, "proj", 4, c4))
                for tt in range(4 * c4, 4 * c4 + 4):
                    items.append((c4, "v", tt))
            for hp in range(1, 4):
                for c4 in range(TC):
                    items.append((hp * 4 + c4 - 1, "proj", hp, c4))
                    items.append((hp * 4 + c4 - 1, "proj", hp + 4, c4))
            items.sort(key=lambda it: it[0])
            fillers = deque(items)

            deficit = 0.0

            def run_item(it):
                if it[1] == "v":
                    v_tile(it[2])
                elif it[1] == "proj":
                    proj_rope(it[2], it[3])
                else:
                    out_proj(it[2])

            def pop_by_deficit():
                nonlocal deficit
                while fillers and deficit >= ITEM_COST[fillers[0][1]]:
                    it = fillers.popleft()
                    deficit -= ITEM_COST[it[1]]
                    run_item(it)

            # bootstrap: only block (0,0)'s dependencies
            proj_rope(0, 0, use_big=True)
            proj_rope(4, 0, use_big=True)
            for tt in range(4):
                v_tile(tt)

            for hp in range(4):
                for qc in range(TC):
                    bidx = hp * 4 + qc
                    # dependencies of this attention block must be emitted
                    while fillers and fillers[0][0] <= bidx:
                        run_item(fillers.popleft())
                    qsl = slice(qc * 512, (qc + 1) * 512)
                    jmax = 4 * qc + 3
                    pv0 = ppv.tile([65, 512], dt.float32, tag="pv")
                    pv1 = ppv.tile([65, 512], dt.float32, tag="pv")
                    for j in range(jmax + 1):
                        qs = max(0, j * 128 - qc * 512)
                        w = 512 - qs
                        # both parities' scores side by side in one 2-bank tile
                        big = pb.tile([128, 1024], dt.float32, tag="big")
                        for par in range(2):
                            kb = par * 64
                            nc.tensor.matmul(
                                big[:, par * 512 + qs:par * 512 + 512],
                                kT_sb[kb:kb + 64, hp, j * 128:(j + 1) * 128],
                                qT_sb[kb:kb + 64, hp, qc * 512 + qs:(qc + 1) * 512],
                                start=True, stop=True,
                            )
                        ex = ep.tile([128, 1024], dts, tag="ex")
                        big_v = big.rearrange("p (two q) -> p two q", two=2)
                        ex_v = ex.rearrange("p (two q) -> p two q", two=2)
                        nc.scalar.activation(
                            ex_v[:, :, qs:512], big_v[:, :, qs:512],
                            mybir.ActivationFunctionType.Exp,
                            bias=0.0, scale=0.125,
                        )
                        if qs > 0 or j * 128 == qc * 512:
                            # diagonal tile: zero the strictly-upper part
                            nc.vector.tensor_tensor(
                                ex_v[:, :, qs:qs + 128],
                                ex_v[:, :, qs:qs + 128],
                                tri_sb[:, None, :].to_broadcast((128, 2, 128)),
                                mybir.AluOpType.mult,
                            )
                        for par in range(2):
                            h = 2 * hp + par
                            pv = pv0 if par == 0 else pv1
                            nc.tensor.matmul(
                                pv[:, qs:512],
                                v_sb[:, j, h * 65:(h + 1) * 65],
                                ex[:, par * 512 + qs:par * 512 + 512],
                                start=(j == 0), stop=(j == jmax),
                            )
                        deficit += DEFICIT_PER_J
                        pop_by_deficit()
                    for par in range(2):
                        h = 2 * hp + par
                        kb = par * 64
                        pv = pv0 if par == 0 else pv1
                        rinv = np_.tile([1, 512], dt.float32, tag="rinv")
                        nc.vector.reciprocal(rinv[0:1, :], pv[64:65, :])
                        rb = np_.tile([64, 512], dt.float32, tag="rb")
                        nc.gpsimd.partition_broadcast(rb[:], rinv[0:1, :])
                        nc.vector.tensor_mul(
                            y_sb[kb:kb + 64, hp, qsl], pv[0:64, :], rb[:],
                        )
                    deficit += BLOCK_BONUS
                    if hp == 3:
                        # this chunk's y is final for all heads: stream out-proj
                        for tt in range(4 * qc, 4 * qc + 4):
                            fillers.append((16, "out", tt))
            while fillers:
                run_item(fillers.popleft())

    nc.compile()
    return nc


def _prep_core_inputs(x, W_qkv, b_qkv, W_out, g):
    """Host-side shard prep for head-group g (features g*512:(g+1)*512)."""
    fs = slice(g * 512, (g + 1) * 512)
    Wq = W_qkv[:, 0:1024][:, fs]          # [1024, 512]
    Wk = W_qkv[:, 1024:2048][:, fs]
    Wv_ = W_qkv[:, 2048:3072][:, fs]
    bq = b_qkv[0:1024][fs]
    bk = b_qkv[1024:2048][fs]
    bv_ = b_qkv[2048:3072][fs]

    Wqk_np = np.concatenate([Wq, Wk], axis=1)        # [1024, 1024]
    # [fs, p, ks, col]
    Wqk_np = Wqk_np.reshape(KSUB, 128, 8, 128).transpose(2, 1, 0, 3)
    Wv_np = Wv_.reshape(KSUB, 128, 512).transpose(1, 0, 2)
    Wo_np = W_out[fs, :].reshape(4, 128, 1024).transpose(1, 0, 2)
    bqk_np = np.concatenate([bq, bk]).reshape(8, 128).T.copy()   # [128, 8]
    bv_np = np.broadcast_to(bv_[None, :], (128, 512)).copy()

    return {
        "Wqk": np.ascontiguousarray(Wqk_np).astype(bf16),
        "Wv": np.ascontiguousarray(Wv_np).astype(bf16),
        "Wo": np.ascontiguousarray(Wo_np).astype(bf16),
        "bqk": np.ascontiguousarray(bqk_np).astype(np.float32),
        "bv": bv_np.astype(np.float32),
    }


def _shared_inputs():
    # rotation matrix: (R q)[d] = -q[d+32] for d<32, q[d-32] for 32<=d<64
    R64 = np.zeros((64, 64), dtype=np.float32)
    for d in range(32):
        R64[d, d + 32] = -1.0
        R64[d + 32, d] = 1.0
    R128 = np.zeros((128, 128), dtype=np.float32)
    R128[0:64, 0:64] = R64
    R128[64:128, 64:128] = R64
    RT_np = R128.T.copy()

    inv_freq = 1.0 / (10000.0 ** (np.arange(0, HD, 2, dtype=np.float32) / HD))
    t = np.arange(T, dtype=np.float32)
    freqs = np.outer(t, inv_freq)                     # [T, 32]
    p = np.arange(128)
    cos_np = np.cos(freqs[:, p % 32]).T.copy()        # [128, T]
    sin_np = np.sin(freqs[:, p % 32]).T.copy()

    tri_np = np.tril(np.ones((128, 128), dtype=np.float32)).T.copy()  # kk<=qq

    return {
        "RT": RT_np.astype(bf16),
        "cosd": np.ascontiguousarray(
            cos_np.reshape(128, TC, 512).transpose(1, 0, 2)).astype(bf16),
        "sind": np.ascontiguousarray(
            sin_np.reshape(128, TC, 512).transpose(1, 0, 2)).astype(bf16),
        "trid": np.ascontiguousarray(tri_np).astype(bf16),
    }


def run(x, W_qkv, b_qkv, W_out, b_out, trace=False):
    global _compiled
    if _compiled is None:
        _compiled = _build()
    nc = _compiled

    shared = _shared_inputs()
    group_inp = [_prep_core_inputs(x, W_qkv, b_qkv, W_out, g) for g in range(2)]

    in_maps = []
    for core in range(N_CORES):
        b, g = core // 2, core % 2
        # [c4, p, ks, q]
        xT_np = (x[b].reshape(TC, 512, KSUB, 128).transpose(0, 3, 2, 1))
        m = {"xT": np.ascontiguousarray(xT_np).astype(bf16)}
        m.update(group_inp[g])
        m.update(shared)
        in_maps.append(m)

    res = run_bass_kernel_spmd(
        nc, in_maps, core_ids=list(range(N_CORES)), trace=trace,
        stitch_traces=trace,
    )
    outp = np.empty((B, T, C), dtype=np.float32)
    for b in range(B):
        outp[b] = (res.results[2 * b]["out"] + res.results[2 * b + 1]["out"]
                   + b_out[None, :])
    return outp, res


def kernel(x, W_qkv, b_qkv, W_out, b_out):
    x = np.asarray(x, dtype=np.float32)
    W_qkv = np.asarray(W_qkv, dtype=np.float32)
    b_qkv = np.asarray(b_qkv, dtype=np.float32)
    W_out = np.asarray(W_out, dtype=np.float32)
    b_out = np.asarray(b_out, dtype=np.float32)
    outp, _ = run(x, W_qkv, b_qkv, W_out, b_out, trace=False)
    return outp



# revision 13
# speedup vs baseline: 1.0482x; 1.0291x over previous
"""Causal self-attention (B=4, T=2048, D=1024, H=16) on 8 TRN2 NeuronCores.

Sharding: core c handles batch b = c // 2 and head-group g = c % 2
(8 heads = 512 of the 1024 feature dims). Each core:
  1. QKV projection for its head-group's columns. q, k are produced
     TRANSPOSED ([feat, tok], feature dim on partitions) so they feed the
     attention matmuls directly; v is produced natural ([tok, feat]) so it
     is the PV stationary operand.
  2. RoPE via a PE rotation matmul (rotate_half as a constant 128x128
     block-diagonal permutation) + DVE combine with cos/sin.
  3. Causal attention with scores in [k, q] orientation: exp without
     max-subtraction (scores/8 are O(1); fp32/bf16 safe), row-sum obtained
     free via a ones-column appended to v (PV matmul M=65: rows 0-63 = y,
     row 64 = softmax denominator).
  4. Late softmax normalization (reciprocal + gpsimd partition-broadcast),
     then the partial output projection with its 512 rows of W_out.
Host sums the two head-group partials per batch and adds b_out.

All matmuls run in bf16 (fp32 matmul is 1/4 rate on the PE); softmax
statistics accumulate in fp32 PSUM.
"""

import numpy as np
import ml_dtypes

import concourse.tile as tile
from concourse import bacc, mybir
from concourse.bass_utils import run_bass_kernel_spmd

dt = mybir.dt
bf16 = ml_dtypes.bfloat16

B, T, C = 4, 2048, 1024
H, HD = 16, 64
N_CORES = 8
HPC = 8          # heads per core
FPC = H // 2 * HD // 8 * 8 // 2 * 2  # = 512 features per core (q, k, v each)
KSUB = C // 128  # 8 contraction subtiles
TT = T // 128    # 16 token tiles
TC = T // 512    # 4 token chunks

_compiled = None


def _build():
    nc = bacc.Bacc()
    dts = dt.bfloat16

    xT = nc.dram_tensor("xT", [TC, 128, KSUB, 512], dts, kind="ExternalInput")
    Wqk = nc.dram_tensor("Wqk", [8, 128, KSUB, 128], dts, kind="ExternalInput")
    Wv = nc.dram_tensor("Wv", [128, KSUB, 512], dts, kind="ExternalInput")
    Wo = nc.dram_tensor("Wo", [128, 4, 1024], dts, kind="ExternalInput")
    bqk = nc.dram_tensor("bqk", [128, 8], dt.float32, kind="ExternalInput")
    bv = nc.dram_tensor("bv", [128, 512], dt.float32, kind="ExternalInput")
    RT = nc.dram_tensor("RT", [128, 128], dts, kind="ExternalInput")
    cosd = nc.dram_tensor("cosd", [TC, 128, 512], dts, kind="ExternalInput")
    sind = nc.dram_tensor("sind", [TC, 128, 512], dts, kind="ExternalInput")
    trid = nc.dram_tensor("trid", [128, 128], dts, kind="ExternalInput")
    out = nc.dram_tensor("out", [T, C], dt.float32, kind="ExternalOutput")

    with tile.TileContext(nc) as tc:
        with (
            tc.tile_pool(name="weights", bufs=1) as wp,
            tc.tile_pool(name="acts", bufs=1) as ap,
            tc.tile_pool(name="scratch", bufs=3) as sp,
            tc.tile_pool(name="exps", bufs=10) as ep,
            tc.tile_pool(name="norm", bufs=2) as np_,
            tc.tile_pool(name="outs", bufs=6) as op,
            tc.tile_pool(name="psum", bufs=2, space="PSUM") as pp,
            tc.tile_pool(name="psum_big", bufs=2, space="PSUM") as pb,
            tc.tile_pool(name="psum_pv", bufs=2, space="PSUM") as ppv,
        ):
            xT_sb = wp.tile([128, KSUB, T], dts)
            Wqk_sb = wp.tile([128, 8, KSUB, 128], dts)
            Wv_sb = wp.tile([128, KSUB, 512], dts)
            Wo_sb = wp.tile([128, 4, 1024], dts)
            bqk_sb = wp.tile([128, 8], dt.float32)
            bv_sb = wp.tile([128, 512], dt.float32)
            RT_sb = wp.tile([128, 128], dts)
            cos_sb = wp.tile([128, T], dts)
            sin_sb = wp.tile([128, T], dts)
            tri_sb = wp.tile([128, 128], dts)

            xTv = xT_sb.rearrange("p k (c q) -> p k c q", c=TC)

            def load_xT(c4):
                nc.sync.dma_start(xTv[:, :, c4, :].rearrange("p k q -> p k q"),
                                  xT[c4])

            def load_wqk(fs):
                nc.sync.dma_start(Wqk_sb[:, fs], Wqk[fs])

            # first-needed data up front in fine grain so the first proj
            # matmuls can start ~2us in, the rest behind it (all DMAs share
            # one serialized engine pool, so order = priority)
            cosv = cos_sb.rearrange("p (c q) -> p c q", c=TC)
            sinv = sin_sb.rearrange("p (c q) -> p c q", c=TC)
            load_wqk(0)
            for ks2 in range(0, KSUB, 2):  # chunk-0 tokens, 2 k-subtiles apiece
                nc.sync.dma_start(xTv[:, ks2:ks2 + 2, 0, :], xT[0, :, ks2:ks2 + 2, :])
            load_wqk(4)
            nc.sync.dma_start(bqk_sb[:], bqk[:])
            nc.sync.dma_start(RT_sb[:], RT[:])
            nc.sync.dma_start(cosv[:, 0], cosd[0])
            nc.sync.dma_start(sinv[:, 0], sind[0])
            nc.sync.dma_start(Wv_sb[:], Wv[:])
            nc.sync.dma_start(bv_sb[:], bv[:])
            nc.sync.dma_start(tri_sb[:], trid[:])
            for c4 in range(1, TC):
                load_xT(c4)
                nc.sync.dma_start(cosv[:, c4], cosd[c4])
                nc.sync.dma_start(sinv[:, c4], sind[c4])
            for fs in (1, 5, 2, 6, 3, 7):
                load_wqk(fs)
            nc.sync.dma_start(Wo_sb[:], Wo[:])

            qT_sb = ap.tile([128, 4, T], dts)   # rope'd q, [feat, tok]
            kT_sb = ap.tile([128, 4, T], dts)   # rope'd k, [feat, tok]
            v_sb = ap.tile([128, TT, 8 * 65], dts)  # v natural + ones col per head
            y_sb = ap.tile([128, 4, T], dts)    # normalized attention out (lhsT)

            # ones columns of v (col 64 of each head's 65-wide block)
            v_heads = v_sb.rearrange("p t (h f) -> p t h f", h=8)
            nc.vector.memset(v_heads[:, :, :, 64], 1.0)

            # PE pre-warm: the tensor engine ramps to full clock only after
            # 3us of execution; burn that ramp on dummy matmuls over a
            # memset tile while the first input DMAs are still in flight so
            # the real matmuls start at full speed.
            warm_sb = wp.tile([128, 64], dts)
            nc.vector.memset(warm_sb[:], 0.0)
            warm_ps = pp.tile([128, 512], dt.float32, tag="ps512")
            for wi in range(10):
                nc.tensor.matmul(warm_ps[0:64, 0:64], warm_sb[:, 0:64],
                                 warm_sb[:], start=True, stop=True)

            # ---- fine-grained work emitters -----------------------------
            def v_tile(tt):
                psv = pp.tile([128, 512], dt.float32, tag="ps512")
                for ks in range(KSUB):
                    nc.tensor.matmul(
                        psv[:],
                        xT_sb[:, ks, tt * 128:(tt + 1) * 128],
                        Wv_sb[:, ks, :],
                        start=(ks == 0), stop=(ks == KSUB - 1),
                    )
                nc.vector.tensor_add(
                    v_heads[:, tt, :, 0:64],
                    psv[:].rearrange("p (h f) -> p h f", h=8),
                    bv_sb[:].rearrange("p (h f) -> p h f", h=8),
                )

            def proj_rope(fs, c4, use_big=False):
                tsl = slice(c4 * 512, (c4 + 1) * 512)
                if use_big:
                    # bootstrap: attention pools are idle, borrow a big tile
                    bigt = pb.tile([128, 1024], dt.float32, tag="big")
                    ps, rps = bigt[:, 0:512], bigt[:, 512:1024]
                else:
                    ps = pp.tile([128, 512], dt.float32, tag="ps512")
                    rps = pp.tile([128, 512], dt.float32, tag="ps512")
                for ks in range(KSUB):
                    nc.tensor.matmul(
                        ps[:],
                        Wqk_sb[:, fs, ks, :],
                        xT_sb[:, ks, tsl],
                        start=(ks == 0), stop=(ks == KSUB - 1),
                    )
                qb = sp.tile([128, 512], dt.float32, tag="qb")
                nc.vector.tensor_scalar_add(qb[:], ps[:], bqk_sb[:, fs:fs + 1])
                u = sp.tile([128, 512], dts, tag="u")
                nc.vector.tensor_mul(u[:], qb[:], sin_sb[:, tsl])
                w = sp.tile([128, 512], dt.float32, tag="w")
                nc.vector.tensor_mul(w[:], qb[:], cos_sb[:, tsl])
                nc.tensor.matmul(rps[:], RT_sb[:], u[:], start=True, stop=True)
                dst = qT_sb if fs < 4 else kT_sb
                nc.vector.tensor_add(dst[:, fs % 4, tsl], w[:], rps[:])

            def out_proj(tt):
                for n2 in range(2):
                    po = pp.tile([128, 512], dt.float32, tag="ps512")
                    for s in range(4):
                        nc.tensor.matmul(
                            po[:],
                            y_sb[:, s, tt * 128:(tt + 1) * 128],
                            Wo_sb[:, s, n2 * 512:(n2 + 1) * 512],
                            start=(s == 0), stop=(s == 3),
                        )
                    ost = op.tile([128, 512], dt.float32, tag="ost")
                    nc.vector.tensor_copy(ost[:], po[:])
                    nc.sync.dma_start(
                        out[tt * 128:(tt + 1) * 128, n2 * 512:(n2 + 1) * 512],
                        ost[:],
                    )

            from collections import deque

            # Filler work items (qkv projections, v tiles, out projections)
            # streamed into the attention pipeline's PE gaps.  The attention
            # inner loop is Activation-paced (exp ~1040ns vs 4 matmuls
            # ~850ns per k-tile step), so the PE has a ~300-500ns deficit
            # per step that filler matmuls must cover; pacing them evenly
            # across ALL 160 steps (instead of draining greedily up front)
            # is what keeps the PE busy end to end.
            # Item = (deadline_block, kind, *args); FIFO is deadline-sorted.
            PE_MM = 213.0  # ns per 512-row bf16 matmul at full clock
            ITEM_COST = {"v": 8 * PE_MM, "proj": 9 * PE_MM, "out": 8 * PE_MM}
            DEFICIT_PER_J = 185.0   # Act busy excess over PE per k-tile step
            DIAG_EXTRA = 600.0      # exp->mask->pv latency on diagonal tiles
            BLOCK_BONUS = 600.0

            # proj items get a one-block-earlier deadline: the block's first
            # scores read qT/kT through the proj's DVE rope chain (~2.3us
            # after its last matmul), so draining at the consuming block
            # stalls the PE on DVE.  v items feed pv directly (short dep) and
            # can drain just in time.
            items = []
            for c4 in range(1, TC):  # head-pair 0's remaining chunks + v
                items.append((c4 - 1, "proj", 0, c4))
                items.append((c4 - 1, "proj", 4, c4))
                for tt in range(4 * c4, 4 * c4 + 4):
                    items.append((c4, "v", tt))
            for hp in range(1, 4):
                for c4 in range(TC):
                    items.append((hp * 4 + c4 - 1, "proj", hp, c4))
                    items.append((hp * 4 + c4 - 1, "proj", hp + 4, c4))
            items.sort(key=lambda it: it[0])
            fillers = deque(items)

            deficit = 0.0

            def run_item(it):
                if it[1] == "v":
                    v_tile(it[2])
                elif it[1] == "proj":
                    proj_rope(it[2], it[3])
                else:
                    out_proj(it[2])

            def pop_by_deficit():
                nonlocal deficit
                while fillers and deficit >= ITEM_COST[fillers[0][1]]:
                    it = fillers.popleft()
                    deficit -= ITEM_COST[it[1]]
                    run_item(it)

            # bootstrap: only block (0,0)'s dependencies
            proj_rope(0, 0, use_big=True)
            proj_rope(4, 0, use_big=True)
            for tt in range(4):
                v_tile(tt)

            for hp in range(4):
                for qc in range(TC):
                    bidx = hp * 4 + qc
                    # dependencies of this attention block must be emitted
                    while fillers and fillers[0][0] <= bidx:
                        run_item(fillers.popleft())
                    qsl = slice(qc * 512, (qc + 1) * 512)
                    jmax = 4 * qc + 3
                    pv0 = ppv.tile([65, 512], dt.float32, tag="pv")
                    pv1 = ppv.tile([65, 512], dt.float32, tag="pv")
                    for j in range(jmax + 1):
                        qs = max(0, j * 128 - qc * 512)
                        w = 512 - qs
                        # both parities' scores side by side in one 2-bank tile
                        big = pb.tile([128, 1024], dt.float32, tag="big")
                        for par in range(2):
                            kb = par * 64
                            nc.tensor.matmul(
                                big[:, par * 512 + qs:par * 512 + 512],
                                kT_sb[kb:kb + 64, hp, j * 128:(j + 1) * 128],
                                qT_sb[kb:kb + 64, hp, qc * 512 + qs:(qc + 1) * 512],
                                start=True, stop=True,
                            )
                        ex = ep.tile([128, 1024], dts, tag="ex")
                        big_v = big.rearrange("p (two q) -> p two q", two=2)
                        ex_v = ex.rearrange("p (two q) -> p two q", two=2)
                        nc.scalar.activation(
                            ex_v[:, :, qs:512], big_v[:, :, qs:512],
                            mybir.ActivationFunctionType.Exp,
                            bias=0.0, scale=0.125,
                        )
                        if qs > 0 or j * 128 == qc * 512:
                            # diagonal tile: zero the strictly-upper part
                            nc.vector.tensor_tensor(
                                ex_v[:, :, qs:qs + 128],
                                ex_v[:, :, qs:qs + 128],
                                tri_sb[:, None, :].to_broadcast((128, 2, 128)),
                                mybir.AluOpType.mult,
                            )
                        for par in range(2):
                            h = 2 * hp + par
                            pv = pv0 if par == 0 else pv1
                            nc.tensor.matmul(
                                pv[:, qs:512],
                                v_sb[:, j, h * 65:(h + 1) * 65],
                                ex[:, par * 512 + qs:par * 512 + 512],
                                start=(j == 0), stop=(j == jmax),
                            )
                        deficit += DEFICIT_PER_J
                        if qs > 0 or j * 128 == qc * 512:
                            deficit += DIAG_EXTRA
                        pop_by_deficit()
                    for par in range(2):
                        h = 2 * hp + par
                        kb = par * 64
                        pv = pv0 if par == 0 else pv1
                        rinv = np_.tile([1, 512], dt.float32, tag="rinv")
                        nc.vector.reciprocal(rinv[0:1, :], pv[64:65, :])
                        rb = np_.tile([64, 512], dt.float32, tag="rb")
                        nc.gpsimd.partition_broadcast(rb[:], rinv[0:1, :])
                        nc.vector.tensor_mul(
                            y_sb[kb:kb + 64, hp, qsl], pv[0:64, :], rb[:],
                        )
                    deficit += BLOCK_BONUS
                    if hp == 3:
                        # this chunk's y is final for all heads: stream out-proj
                        for tt in range(4 * qc, 4 * qc + 4):
                            fillers.append((16, "out", tt))
            while fillers:
                run_item(fillers.popleft())

    nc.compile()
    return nc


def _prep_core_inputs(x, W_qkv, b_qkv, W_out, g):
    """Host-side shard prep for head-group g (features g*512:(g+1)*512)."""
    fs = slice(g * 512, (g + 1) * 512)
    Wq = W_qkv[:, 0:1024][:, fs]          # [1024, 512]
    Wk = W_qkv[:, 1024:2048][:, fs]
    Wv_ = W_qkv[:, 2048:3072][:, fs]
    bq = b_qkv[0:1024][fs]
    bk = b_qkv[1024:2048][fs]
    bv_ = b_qkv[2048:3072][fs]

    Wqk_np = np.concatenate([Wq, Wk], axis=1)        # [1024, 1024]
    # [fs, p, ks, col]
    Wqk_np = Wqk_np.reshape(KSUB, 128, 8, 128).transpose(2, 1, 0, 3)
    Wv_np = Wv_.reshape(KSUB, 128, 512).transpose(1, 0, 2)
    Wo_np = W_out[fs, :].reshape(4, 128, 1024).transpose(1, 0, 2)
    bqk_np = np.concatenate([bq, bk]).reshape(8, 128).T.copy()   # [128, 8]
    bv_np = np.broadcast_to(bv_[None, :], (128, 512)).copy()

    return {
        "Wqk": np.ascontiguousarray(Wqk_np).astype(bf16),
        "Wv": np.ascontiguousarray(Wv_np).astype(bf16),
        "Wo": np.ascontiguousarray(Wo_np).astype(bf16),
        "bqk": np.ascontiguousarray(bqk_np).astype(np.float32),
        "bv": bv_np.astype(np.float32),
    }


def _shared_inputs():
    # rotation matrix: (R q)[d] = -q[d+32] for d<32, q[d-32] for 32<=d<64
    R64 = np.zeros((64, 64), dtype=np.float32)
    for d in range(32):
        R64[d, d + 32] = -1.0
        R64[d + 32, d] = 1.0
    R128 = np.zeros((128, 128), dtype=np.float32)
    R128[0:64, 0:64] = R64
    R128[64:128, 64:128] = R64
    RT_np = R128.T.copy()

    inv_freq = 1.0 / (10000.0 ** (np.arange(0, HD, 2, dtype=np.float32) / HD))
    t = np.arange(T, dtype=np.float32)
    freqs = np.outer(t, inv_freq)                     # [T, 32]
    p = np.arange(128)
    cos_np = np.cos(freqs[:, p % 32]).T.copy()        # [128, T]
    sin_np = np.sin(freqs[:, p % 32]).T.copy()

    tri_np = np.tril(np.ones((128, 128), dtype=np.float32)).T.copy()  # kk<=qq

    return {
        "RT": RT_np.astype(bf16),
        "cosd": np.ascontiguousarray(
            cos_np.reshape(128, TC, 512).transpose(1, 0, 2)).astype(bf16),
        "sind": np.ascontiguousarray(
            sin_np.reshape(128, TC, 512).transpose(1, 0, 2)).astype(bf16),
        "trid": np.ascontiguousarray(tri_np).astype(bf16),
    }


def run(x, W_qkv, b_qkv, W_out, b_out, trace=False):
    global _compiled
    if _compiled is None:
        _compiled = _build()
    nc = _compiled

    shared = _shared_inputs()
    group_inp = [_prep_core_inputs(x, W_qkv, b_qkv, W_out, g) for g in range(2)]

    in_maps = []
    for core in range(N_CORES):
        b, g = core // 2, core % 2
        # [c4, p, ks, q]
        xT_np = (x[b].reshape(TC, 512, KSUB, 128).transpose(0, 3, 2, 1))
        m = {"xT": np.ascontiguousarray(xT_np).astype(bf16)}
        m.update(group_inp[g])
        m.update(shared)
        in_maps.append(m)

    res = run_bass_kernel_spmd(
        nc, in_maps, core_ids=list(range(N_CORES)), trace=trace,
        stitch_traces=trace,
    )
    outp = np.empty((B, T, C), dtype=np.float32)
    for b in range(B):
        outp[b] = (res.results[2 * b]["out"] + res.results[2 * b + 1]["out"]
                   + b_out[None, :])
    return outp, res


def kernel(x, W_qkv, b_qkv, W_out, b_out):
    x = np.asarray(x, dtype=np.float32)
    W_qkv = np.asarray(W_qkv, dtype=np.float32)
    b_qkv = np.asarray(b_qkv, dtype=np.float32)
    W_out = np.asarray(W_out, dtype=np.float32)
    b_out = np.asarray(b_out, dtype=np.float32)
    outp, _ = run(x, W_qkv, b_qkv, W_out, b_out, trace=False)
    return outp



# revision 14
# speedup vs baseline: 1.0521x; 1.0037x over previous
"""Causal self-attention (B=4, T=2048, D=1024, H=16) on 8 TRN2 NeuronCores.

Sharding: core c handles batch b = c // 2 and head-group g = c % 2
(8 heads = 512 of the 1024 feature dims). Each core:
  1. QKV projection for its head-group's columns. q, k are produced
     TRANSPOSED ([feat, tok], feature dim on partitions) so they feed the
     attention matmuls directly; v is produced natural ([tok, feat]) so it
     is the PV stationary operand.
  2. RoPE via a PE rotation matmul (rotate_half as a constant 128x128
     block-diagonal permutation) + DVE combine with cos/sin.
  3. Causal attention with scores in [k, q] orientation: exp without
     max-subtraction (scores/8 are O(1); fp32/bf16 safe), row-sum obtained
     free via a ones-column appended to v (PV matmul M=65: rows 0-63 = y,
     row 64 = softmax denominator).
  4. Late softmax normalization (reciprocal + gpsimd partition-broadcast),
     then the partial output projection with its 512 rows of W_out.
Host sums the two head-group partials per batch and adds b_out.

All matmuls run in bf16 (fp32 matmul is 1/4 rate on the PE); softmax
statistics accumulate in fp32 PSUM.
"""

import numpy as np
import ml_dtypes

import concourse.tile as tile
from concourse import bacc, mybir
from concourse.bass_utils import run_bass_kernel_spmd

dt = mybir.dt
bf16 = ml_dtypes.bfloat16

B, T, C = 4, 2048, 1024
H, HD = 16, 64
N_CORES = 8
HPC = 8          # heads per core
FPC = H // 2 * HD // 8 * 8 // 2 * 2  # = 512 features per core (q, k, v each)
KSUB = C // 128  # 8 contraction subtiles
TT = T // 128    # 16 token tiles
TC = T // 512    # 4 token chunks

_compiled = None


def _build():
    nc = bacc.Bacc()
    dts = dt.bfloat16

    xT = nc.dram_tensor("xT", [TC, 128, KSUB, 512], dts, kind="ExternalInput")
    Wqk = nc.dram_tensor("Wqk", [8, 128, KSUB, 128], dts, kind="ExternalInput")
    Wv = nc.dram_tensor("Wv", [128, KSUB, 512], dts, kind="ExternalInput")
    Wo = nc.dram_tensor("Wo", [128, 4, 1024], dts, kind="ExternalInput")
    bqk = nc.dram_tensor("bqk", [128, 8], dt.float32, kind="ExternalInput")
    bv = nc.dram_tensor("bv", [128, 512], dt.float32, kind="ExternalInput")
    RT = nc.dram_tensor("RT", [128, 128], dts, kind="ExternalInput")
    cosd = nc.dram_tensor("cosd", [TC, 128, 512], dts, kind="ExternalInput")
    sind = nc.dram_tensor("sind", [TC, 128, 512], dts, kind="ExternalInput")
    trid = nc.dram_tensor("trid", [128, 128], dts, kind="ExternalInput")
    out = nc.dram_tensor("out", [T, C], dt.float32, kind="ExternalOutput")

    with tile.TileContext(nc) as tc:
        with (
            tc.tile_pool(name="weights", bufs=1) as wp,
            tc.tile_pool(name="acts", bufs=1) as ap,
            tc.tile_pool(name="scratch", bufs=3) as sp,
            tc.tile_pool(name="exps", bufs=10) as ep,
            tc.tile_pool(name="norm", bufs=2) as np_,
            tc.tile_pool(name="outs", bufs=6) as op,
            tc.tile_pool(name="psum", bufs=2, space="PSUM") as pp,
            tc.tile_pool(name="psum_big", bufs=2, space="PSUM") as pb,
            tc.tile_pool(name="psum_pv", bufs=2, space="PSUM") as ppv,
        ):
            xT_sb = wp.tile([128, KSUB, T], dts)
            Wqk_sb = wp.tile([128, 8, KSUB, 128], dts)
            Wv_sb = wp.tile([128, KSUB, 512], dts)
            Wo_sb = wp.tile([128, 4, 1024], dts)
            bqk_sb = wp.tile([128, 8], dt.float32)
            bv_sb = wp.tile([128, 512], dt.float32)
            RT_sb = wp.tile([128, 128], dts)
            cos_sb = wp.tile([128, T], dts)
            sin_sb = wp.tile([128, T], dts)
            tri_sb = wp.tile([128, 128], dts)

            xTv = xT_sb.rearrange("p k (c q) -> p k c q", c=TC)

            def load_xT(c4):
                nc.sync.dma_start(xTv[:, :, c4, :].rearrange("p k q -> p k q"),
                                  xT[c4])

            def load_wqk(fs):
                nc.sync.dma_start(Wqk_sb[:, fs], Wqk[fs])

            # first-needed data up front in fine grain so the first proj
            # matmuls can start ~2us in, the rest behind it (all DMAs share
            # one serialized engine pool, so order = priority)
            cosv = cos_sb.rearrange("p (c q) -> p c q", c=TC)
            sinv = sin_sb.rearrange("p (c q) -> p c q", c=TC)
            load_wqk(0)
            for ks2 in range(0, KSUB, 2):  # chunk-0 tokens, 2 k-subtiles apiece
                nc.sync.dma_start(xTv[:, ks2:ks2 + 2, 0, :], xT[0, :, ks2:ks2 + 2, :])
            load_wqk(4)
            nc.sync.dma_start(bqk_sb[:], bqk[:])
            nc.sync.dma_start(RT_sb[:], RT[:])
            nc.sync.dma_start(cosv[:, 0], cosd[0])
            nc.sync.dma_start(sinv[:, 0], sind[0])
            nc.sync.dma_start(Wv_sb[:], Wv[:])
            nc.sync.dma_start(bv_sb[:], bv[:])
            nc.sync.dma_start(tri_sb[:], trid[:])
            for c4 in range(1, TC):
                load_xT(c4)
                nc.sync.dma_start(cosv[:, c4], cosd[c4])
                nc.sync.dma_start(sinv[:, c4], sind[c4])
            for fs in (1, 5, 2, 6, 3, 7):
                load_wqk(fs)
            nc.sync.dma_start(Wo_sb[:], Wo[:])

            qT_sb = ap.tile([128, 4, T], dts)   # rope'd q, [feat, tok]
            kT_sb = ap.tile([128, 4, T], dts)   # rope'd k, [feat, tok]
            v_sb = ap.tile([128, TT, 8 * 65], dts)  # v natural + ones col per head
            y_sb = ap.tile([128, 4, T], dts)    # normalized attention out (lhsT)

            # ones columns of v (col 64 of each head's 65-wide block)
            v_heads = v_sb.rearrange("p t (h f) -> p t h f", h=8)
            nc.vector.memset(v_heads[:, :, :, 64], 1.0)

            # PE pre-warm: the tensor engine ramps to full clock only after
            # 3us of execution; burn that ramp on dummy matmuls over a
            # memset tile while the first input DMAs are still in flight so
            # the real matmuls start at full speed.
            warm_sb = wp.tile([128, 64], dts)
            nc.vector.memset(warm_sb[:], 0.0)
            warm_ps = pp.tile([128, 512], dt.float32, tag="ps512")
            for wi in range(10):
                nc.tensor.matmul(warm_ps[0:64, 0:64], warm_sb[:, 0:64],
                                 warm_sb[:], start=True, stop=True)

            # ---- fine-grained work emitters -----------------------------
            def v_tile(tt):
                psv = pp.tile([128, 512], dt.float32, tag="ps512")
                for ks in range(KSUB):
                    nc.tensor.matmul(
                        psv[:],
                        xT_sb[:, ks, tt * 128:(tt + 1) * 128],
                        Wv_sb[:, ks, :],
                        start=(ks == 0), stop=(ks == KSUB - 1),
                    )
                nc.vector.tensor_add(
                    v_heads[:, tt, :, 0:64],
                    psv[:].rearrange("p (h f) -> p h f", h=8),
                    bv_sb[:].rearrange("p (h f) -> p h f", h=8),
                )

            def proj_rope(fs, c4, use_big=False):
                tsl = slice(c4 * 512, (c4 + 1) * 512)
                if use_big:
                    # bootstrap: attention pools are idle, borrow a big tile
                    bigt = pb.tile([128, 1024], dt.float32, tag="big")
                    ps, rps = bigt[:, 0:512], bigt[:, 512:1024]
                else:
                    ps = pp.tile([128, 512], dt.float32, tag="ps512")
                    rps = pp.tile([128, 512], dt.float32, tag="ps512")
                for ks in range(KSUB):
                    nc.tensor.matmul(
                        ps[:],
                        Wqk_sb[:, fs, ks, :],
                        xT_sb[:, ks, tsl],
                        start=(ks == 0), stop=(ks == KSUB - 1),
                    )
                qb = sp.tile([128, 512], dt.float32, tag="qb")
                nc.vector.tensor_scalar_add(qb[:], ps[:], bqk_sb[:, fs:fs + 1])
                u = sp.tile([128, 512], dts, tag="u")
                nc.vector.tensor_mul(u[:], qb[:], sin_sb[:, tsl])
                w = sp.tile([128, 512], dt.float32, tag="w")
                nc.vector.tensor_mul(w[:], qb[:], cos_sb[:, tsl])
                nc.tensor.matmul(rps[:], RT_sb[:], u[:], start=True, stop=True)
                dst = qT_sb if fs < 4 else kT_sb
                nc.vector.tensor_add(dst[:, fs % 4, tsl], w[:], rps[:])

            def out_proj(tt):
                for n2 in range(2):
                    po = pp.tile([128, 512], dt.float32, tag="ps512")
                    for s in range(4):
                        nc.tensor.matmul(
                            po[:],
                            y_sb[:, s, tt * 128:(tt + 1) * 128],
                            Wo_sb[:, s, n2 * 512:(n2 + 1) * 512],
                            start=(s == 0), stop=(s == 3),
                        )
                    ost = op.tile([128, 512], dt.float32, tag="ost")
                    nc.vector.tensor_copy(ost[:], po[:])
                    nc.sync.dma_start(
                        out[tt * 128:(tt + 1) * 128, n2 * 512:(n2 + 1) * 512],
                        ost[:],
                    )

            from collections import deque

            # Filler work items (qkv projections, v tiles, out projections)
            # streamed into the attention pipeline's PE gaps.  The attention
            # inner loop is Activation-paced (exp ~1040ns vs 4 matmuls
            # ~850ns per k-tile step), so the PE has a ~300-500ns deficit
            # per step that filler matmuls must cover; pacing them evenly
            # across ALL 160 steps (instead of draining greedily up front)
            # is what keeps the PE busy end to end.
            # Item = (deadline_block, kind, *args); FIFO is deadline-sorted.
            PE_MM = 213.0  # ns per 512-row bf16 matmul at full clock
            ITEM_COST = {"v": 8 * PE_MM, "proj": 9 * PE_MM, "out": 8 * PE_MM}
            DEFICIT_PER_J = 185.0   # Act busy excess over PE per k-tile step
            DIAG_EXTRA = 600.0      # exp->mask->pv latency on diagonal tiles
            BLOCK_BONUS = 600.0

            # proj items get a one-block-earlier deadline: the block's first
            # scores read qT/kT through the proj's DVE rope chain (~2.3us
            # after its last matmul), so draining at the consuming block
            # stalls the PE on DVE.  v items feed pv directly (short dep) and
            # can drain just in time.
            items = []
            for c4 in range(1, TC):  # head-pair 0's remaining chunks + v
                items.append((c4 - 1, "proj", 0, c4))
                items.append((c4 - 1, "proj", 4, c4))
                for tt in range(4 * c4, 4 * c4 + 4):
                    items.append((c4, "v", tt))
            for hp in range(1, 4):
                for c4 in range(TC):
                    items.append((hp * 4 + c4 - 1, "proj", hp, c4))
                    items.append((hp * 4 + c4 - 1, "proj", hp + 4, c4))
            items.sort(key=lambda it: it[0])
            fillers = deque(items)

            deficit = 0.0

            def run_item(it):
                if it[1] == "v":
                    v_tile(it[2])
                elif it[1] == "proj":
                    proj_rope(it[2], it[3])
                else:
                    out_proj(it[2])

            def pop_by_deficit():
                nonlocal deficit
                while fillers and deficit >= ITEM_COST[fillers[0][1]]:
                    it = fillers.popleft()
                    deficit -= ITEM_COST[it[1]]
                    run_item(it)

            # bootstrap: only block (0,0)'s dependencies
            proj_rope(0, 0, use_big=True)
            proj_rope(4, 0, use_big=True)
            for tt in range(4):
                v_tile(tt)

            for hp in range(4):
                for qc in range(TC):
                    bidx = hp * 4 + qc
                    # dependencies of this attention block must be emitted
                    while fillers and fillers[0][0] <= bidx:
                        run_item(fillers.popleft())
                    qsl = slice(qc * 512, (qc + 1) * 512)
                    jmax = 4 * qc + 3
                    pv0 = ppv.tile([65, 512], dt.float32, tag="pv")
                    pv1 = ppv.tile([65, 512], dt.float32, tag="pv")
                    for j in range(jmax + 1):
                        qs = max(0, j * 128 - qc * 512)
                        w = 512 - qs
                        # both parities' scores side by side in one 2-bank tile
                        big = pb.tile([128, 1024], dt.float32, tag="big")
                        for par in range(2):
                            kb = par * 64
                            nc.tensor.matmul(
                                big[:, par * 512 + qs:par * 512 + 512],
                                kT_sb[kb:kb + 64, hp, j * 128:(j + 1) * 128],
                                qT_sb[kb:kb + 64, hp, qc * 512 + qs:(qc + 1) * 512],
                                start=True, stop=True,
                            )
                        ex = ep.tile([128, 1024], dts, tag="ex")
                        big_v = big.rearrange("p (two q) -> p two q", two=2)
                        ex_v = ex.rearrange("p (two q) -> p two q", two=2)
                        nc.scalar.activation(
                            ex_v[:, :, qs:512], big_v[:, :, qs:512],
                            mybir.ActivationFunctionType.Exp,
                            bias=0.0, scale=0.125,
                        )
                        diag = qs > 0 or j * 128 == qc * 512
                        if diag:
                            # diagonal tile: zero the strictly-upper part
                            nc.vector.tensor_tensor(
                                ex_v[:, :, qs:qs + 128],
                                ex_v[:, :, qs:qs + 128],
                                tri_sb[:, None, :].to_broadcast((128, 2, 128)),
                                mybir.AluOpType.mult,
                            )
                        # fillers go BEFORE the pv matmuls: pv blocks on the
                        # exp (and mask) chain, and blocked instructions fill
                        # the PE's depth-4 wait queue — ready work emitted
                        # behind them could not issue.
                        deficit += DEFICIT_PER_J + (DIAG_EXTRA if diag else 0)
                        pop_by_deficit()
                        for par in range(2):
                            h = 2 * hp + par
                            pv = pv0 if par == 0 else pv1
                            nc.tensor.matmul(
                                pv[:, qs:512],
                                v_sb[:, j, h * 65:(h + 1) * 65],
                                ex[:, par * 512 + qs:par * 512 + 512],
                                start=(j == 0), stop=(j == jmax),
                            )
                    for par in range(2):
                        h = 2 * hp + par
                        kb = par * 64
                        pv = pv0 if par == 0 else pv1
                        rinv = np_.tile([1, 512], dt.float32, tag="rinv")
                        nc.vector.reciprocal(rinv[0:1, :], pv[64:65, :])
                        rb = np_.tile([64, 512], dt.float32, tag="rb")
                        nc.gpsimd.partition_broadcast(rb[:], rinv[0:1, :])
                        nc.vector.tensor_mul(
                            y_sb[kb:kb + 64, hp, qsl], pv[0:64, :], rb[:],
                        )
                    deficit += BLOCK_BONUS
                    if hp == 3:
                        # this chunk's y is final for all heads: stream out-proj
                        for tt in range(4 * qc, 4 * qc + 4):
                            fillers.append((16, "out", tt))
            while fillers:
                run_item(fillers.popleft())

    nc.compile()
    return nc


def _prep_core_inputs(x, W_qkv, b_qkv, W_out, g):
    """Host-side shard prep for head-group g (features g*512:(g+1)*512)."""
    fs = slice(g * 512, (g + 1) * 512)
    Wq = W_qkv[:, 0:1024][:, fs]          # [1024, 512]
    Wk = W_qkv[:, 1024:2048][:, fs]
    Wv_ = W_qkv[:, 2048:3072][:, fs]
    bq = b_qkv[0:1024][fs]
    bk = b_qkv[1024:2048][fs]
    bv_ = b_qkv[2048:3072][fs]

    Wqk_np = np.concatenate([Wq, Wk], axis=1)        # [1024, 1024]
    # [fs, p, ks, col]
    Wqk_np = Wqk_np.reshape(KSUB, 128, 8, 128).transpose(2, 1, 0, 3)
    Wv_np = Wv_.reshape(KSUB, 128, 512).transpose(1, 0, 2)
    Wo_np = W_out[fs, :].reshape(4, 128, 1024).transpose(1, 0, 2)
    bqk_np = np.concatenate([bq, bk]).reshape(8, 128).T.copy()   # [128, 8]
    bv_np = np.broadcast_to(bv_[None, :], (128, 512)).copy()

    return {
        "Wqk": np.ascontiguousarray(Wqk_np).astype(bf16),
        "Wv": np.ascontiguousarray(Wv_np).astype(bf16),
        "Wo": np.ascontiguousarray(Wo_np).astype(bf16),
        "bqk": np.ascontiguousarray(bqk_np).astype(np.float32),
        "bv": bv_np.astype(np.float32),
    }


def _shared_inputs():
    # rotation matrix: (R q)[d] = -q[d+32] for d<32, q[d-32] for 32<=d<64
    R64 = np.zeros((64, 64), dtype=np.float32)
    for d in range(32):
        R64[d, d + 32] = -1.0
        R64[d + 32, d] = 1.0
    R128 = np.zeros((128, 128), dtype=np.float32)
    R128[0:64, 0:64] = R64
    R128[64:128, 64:128] = R64
    RT_np = R128.T.copy()

    inv_freq = 1.0 / (10000.0 ** (np.arange(0, HD, 2, dtype=np.float32) / HD))
    t = np.arange(T, dtype=np.float32)
    freqs = np.outer(t, inv_freq)                     # [T, 32]
    p = np.arange(128)
    cos_np = np.cos(freqs[:, p % 32]).T.copy()        # [128, T]
    sin_np = np.sin(freqs[:, p % 32]).T.copy()

    tri_np = np.tril(np.ones((128, 128), dtype=np.float32)).T.copy()  # kk<=qq

    return {
        "RT": RT_np.astype(bf16),
        "cosd": np.ascontiguousarray(
            cos_np.reshape(128, TC, 512).transpose(1, 0, 2)).astype(bf16),
        "sind": np.ascontiguousarray(
            sin_np.reshape(128, TC, 512).transpose(1, 0, 2)).astype(bf16),
        "trid": np.ascontiguousarray(tri_np).astype(bf16),
    }


def run(x, W_qkv, b_qkv, W_out, b_out, trace=False):
    global _compiled
    if _compiled is None:
        _compiled = _build()
    nc = _compiled

    shared = _shared_inputs()
    group_inp = [_prep_core_inputs(x, W_qkv, b_qkv, W_out, g) for g in range(2)]

    in_maps = []
    for core in range(N_CORES):
        b, g = core // 2, core % 2
        # [c4, p, ks, q]
        xT_np = (x[b].reshape(TC, 512, KSUB, 128).transpose(0, 3, 2, 1))
        m = {"xT": np.ascontiguousarray(xT_np).astype(bf16)}
        m.update(group_inp[g])
        m.update(shared)
        in_maps.append(m)

    res = run_bass_kernel_spmd(
        nc, in_maps, core_ids=list(range(N_CORES)), trace=trace,
        stitch_traces=trace,
    )
    outp = np.empty((B, T, C), dtype=np.float32)
    for b in range(B):
        outp[b] = (res.results[2 * b]["out"] + res.results[2 * b + 1]["out"]
                   + b_out[None, :])
    return outp, res


def kernel(x, W_qkv, b_qkv, W_out, b_out):
    x = np.asarray(x, dtype=np.float32)
    W_qkv = np.asarray(W_qkv, dtype=np.float32)
    b_qkv = np.asarray(b_qkv, dtype=np.float32)
    W_out = np.asarray(W_out, dtype=np.float32)
    b_out = np.asarray(b_out, dtype=np.float32)
    outp, _ = run(x, W_qkv, b_qkv, W_out, b_out, trace=False)
    return outp



# revision 21
# speedup vs baseline: 1.0637x; 1.0110x over previous
"""Causal self-attention (B=4, T=2048, D=1024, H=16) on 8 TRN2 NeuronCores.

Sharding: core c handles batch b = c // 2 and head-group g = c % 2
(8 heads = 512 of the 1024 feature dims). Each core:
  1. QKV projection for its head-group's columns. q, k are produced
     TRANSPOSED ([feat, tok], feature dim on partitions) so they feed the
     attention matmuls directly; v is produced natural ([tok, feat]) so it
     is the PV stationary operand.
  2. RoPE via a PE rotation matmul (rotate_half as a constant 128x128
     block-diagonal permutation) + DVE combine with cos/sin.
  3. Causal attention with scores in [k, q] orientation: exp without
     max-subtraction (scores/8 are O(1); fp32/bf16 safe), row-sum obtained
     free via a ones-column appended to v (PV matmul M=65: rows 0-63 = y,
     row 64 = softmax denominator).
  4. Late softmax normalization (reciprocal + gpsimd partition-broadcast),
     then the partial output projection with its 512 rows of W_out.
Host sums the two head-group partials per batch and adds b_out.

All matmuls run in bf16 (fp32 matmul is 1/4 rate on the PE); softmax
statistics accumulate in fp32 PSUM.
"""

import numpy as np
import ml_dtypes

import concourse.tile as tile
from concourse import bacc, mybir
from concourse.bass_utils import run_bass_kernel_spmd

dt = mybir.dt
bf16 = ml_dtypes.bfloat16

B, T, C = 4, 2048, 1024
H, HD = 16, 64
N_CORES = 8
HPC = 8          # heads per core
FPC = H // 2 * HD // 8 * 8 // 2 * 2  # = 512 features per core (q, k, v each)
KSUB = C // 128  # 8 contraction subtiles
TT = T // 128    # 16 token tiles
TC = T // 512    # 4 token chunks

_compiled = None

# instruction label map for offline timeline analysis (analyze.py)
LABELS = {}
_cur_label = [""]


def _lbl(s):
    _cur_label[0] = s


def _build():
    nc = bacc.Bacc()
    dts = dt.bfloat16

    _orig_name = nc.get_next_instruction_name

    def _named():
        n = _orig_name()
        LABELS[n] = _cur_label[0]
        return n

    nc.get_next_instruction_name = _named

    xT = nc.dram_tensor("xT", [TC, 128, KSUB, 512], dts, kind="ExternalInput")
    Wqk = nc.dram_tensor("Wqk", [8, 128, KSUB, 128], dts, kind="ExternalInput")
    Wv = nc.dram_tensor("Wv", [128, KSUB, 512], dts, kind="ExternalInput")
    Wo = nc.dram_tensor("Wo", [128, 4, 1024], dts, kind="ExternalInput")
    bqk = nc.dram_tensor("bqk", [128, 8], dt.float32, kind="ExternalInput")
    bv = nc.dram_tensor("bv", [128, 512], dt.float32, kind="ExternalInput")
    RT = nc.dram_tensor("RT", [128, 128], dts, kind="ExternalInput")
    cosd = nc.dram_tensor("cosd", [TC, 128, 512], dts, kind="ExternalInput")
    sind = nc.dram_tensor("sind", [TC, 128, 512], dts, kind="ExternalInput")
    trid = nc.dram_tensor("trid", [128, 128], dts, kind="ExternalInput")
    out = nc.dram_tensor("out", [T, C], dt.float32, kind="ExternalOutput")

    with tile.TileContext(nc) as tc:
        with (
            tc.tile_pool(name="weights", bufs=1) as wp,
            tc.tile_pool(name="acts", bufs=1) as ap,
            tc.tile_pool(name="scratch", bufs=3) as sp,
            tc.tile_pool(name="exps", bufs=10) as ep,
            tc.tile_pool(name="norm", bufs=2) as np_,
            tc.tile_pool(name="outs", bufs=6) as op,
            tc.tile_pool(name="psum", bufs=2, space="PSUM") as pp,
            tc.tile_pool(name="psum_big", bufs=4, space="PSUM") as pb,
            tc.tile_pool(name="psum_pv", bufs=2, space="PSUM") as ppv,
        ):
            xT_sb = wp.tile([128, KSUB, T], dts)
            Wqk_sb = wp.tile([128, 8, KSUB, 128], dts)
            Wv_sb = wp.tile([128, KSUB, 512], dts)
            Wo_sb = wp.tile([128, 4, 1024], dts)
            bqk_sb = wp.tile([128, 8], dt.float32)
            bv_sb = wp.tile([128, 512], dt.float32)
            RT_sb = wp.tile([128, 128], dts)
            cos_sb = wp.tile([128, T], dts)
            sin_sb = wp.tile([128, T], dts)
            tri_sb = wp.tile([128, 128], dts)

            xTv = xT_sb.rearrange("p k (c q) -> p k c q", c=TC)

            def load_xT(c4):
                nc.sync.dma_start(xTv[:, :, c4, :].rearrange("p k q -> p k q"),
                                  xT[c4])

            def load_wqk(fs):
                nc.sync.dma_start(Wqk_sb[:, fs], Wqk[fs])

            # first-needed data up front in fine grain so the first proj
            # matmuls can start ~2us in, the rest behind it (all DMAs share
            # one serialized engine pool, so order = priority)
            cosv = cos_sb.rearrange("p (c q) -> p c q", c=TC)
            sinv = sin_sb.rearrange("p (c q) -> p c q", c=TC)
            load_wqk(0)
            for ks2 in range(0, KSUB, 2):  # chunk-0 tokens, 2 k-subtiles apiece
                nc.sync.dma_start(xTv[:, ks2:ks2 + 2, 0, :], xT[0, :, ks2:ks2 + 2, :])
            load_wqk(4)
            nc.sync.dma_start(bqk_sb[:], bqk[:])
            nc.sync.dma_start(RT_sb[:], RT[:])
            nc.sync.dma_start(cosv[:, 0], cosd[0])
            nc.sync.dma_start(sinv[:, 0], sind[0])
            nc.sync.dma_start(Wv_sb[:], Wv[:])
            nc.sync.dma_start(bv_sb[:], bv[:])
            nc.sync.dma_start(tri_sb[:], trid[:])
            for c4 in range(1, TC):
                load_xT(c4)
                nc.sync.dma_start(cosv[:, c4], cosd[c4])
                nc.sync.dma_start(sinv[:, c4], sind[c4])
            for fs in (1, 5, 2, 6, 3, 7):
                load_wqk(fs)
            nc.sync.dma_start(Wo_sb[:], Wo[:])

            qT_sb = ap.tile([128, 4, T], dts)   # rope'd q, [feat, tok]
            kT_sb = ap.tile([128, 4, T], dts)   # rope'd k, [feat, tok]
            v_sb = ap.tile([128, TT, 8 * 65], dts)  # v natural + ones col per head
            y_sb = ap.tile([128, 4, T], dts)    # normalized attention out (lhsT)

            # ones columns of v (col 64 of each head's 65-wide block)
            v_heads = v_sb.rearrange("p t (h f) -> p t h f", h=8)
            nc.vector.memset(v_heads[:, :, :, 64], 1.0)

            # PE pre-warm: the tensor engine ramps to full clock only after
            # 3us of execution; burn that ramp on dummy matmuls over a
            # memset tile while the first input DMAs are still in flight so
            # the real matmuls start at full speed.
            warm_sb = wp.tile([128, 64], dts)
            nc.vector.memset(warm_sb[:], 0.0)
            warm_ps = pp.tile([128, 512], dt.float32, tag="ps512")
            _lbl("warm")
            for wi in range(10):
                nc.tensor.matmul(warm_ps[0:64, 0:64], warm_sb[:, 0:64],
                                 warm_sb[:], start=True, stop=True)

            # ---- fine-grained work emitters -----------------------------
            def v_tile(tt):
                _lbl("v")
                psv = pp.tile([128, 512], dt.float32, tag="ps512")
                for ks in range(KSUB):
                    nc.tensor.matmul(
                        psv[:],
                        xT_sb[:, ks, tt * 128:(tt + 1) * 128],
                        Wv_sb[:, ks, :],
                        start=(ks == 0), stop=(ks == KSUB - 1),
                    )
                nc.vector.tensor_add(
                    v_heads[:, tt, :, 0:64],
                    psv[:].rearrange("p (h f) -> p h f", h=8),
                    bv_sb[:].rearrange("p (h f) -> p h f", h=8),
                )

            def proj_rope(fs, c4, use_big=False):
                _lbl("proj")
                tsl = slice(c4 * 512, (c4 + 1) * 512)
                if use_big:
                    # bootstrap: attention pools are idle, borrow score tiles
                    ps = pb.tile([128, 512], dt.float32, tag="big")
                    rps = pb.tile([128, 512], dt.float32, tag="big")
                else:
                    ps = pp.tile([128, 512], dt.float32, tag="ps512")
                    rps = pp.tile([128, 512], dt.float32, tag="ps512")
                for ks in range(KSUB):
                    nc.tensor.matmul(
                        ps[:],
                        Wqk_sb[:, fs, ks, :],
                        xT_sb[:, ks, tsl],
                        start=(ks == 0), stop=(ks == KSUB - 1),
                    )
                qb = sp.tile([128, 512], dt.float32, tag="qb")
                nc.vector.tensor_scalar_add(qb[:], ps[:], bqk_sb[:, fs:fs + 1])
                u = sp.tile([128, 512], dts, tag="u")
                nc.vector.tensor_mul(u[:], qb[:], sin_sb[:, tsl])
                w = sp.tile([128, 512], dt.float32, tag="w")
                nc.vector.tensor_mul(w[:], qb[:], cos_sb[:, tsl])
                _lbl("rope_mm")
                nc.tensor.matmul(rps[:], RT_sb[:], u[:], start=True, stop=True)
                _lbl("proj_dve")
                dst = qT_sb if fs < 4 else kT_sb
                nc.vector.tensor_add(dst[:, fs % 4, tsl], w[:], rps[:])

            def out_proj(tt):
                _lbl("out")
                for n2 in range(2):
                    po = pp.tile([128, 512], dt.float32, tag="ps512")
                    for s in range(4):
                        nc.tensor.matmul(
                            po[:],
                            y_sb[:, s, tt * 128:(tt + 1) * 128],
                            Wo_sb[:, s, n2 * 512:(n2 + 1) * 512],
                            start=(s == 0), stop=(s == 3),
                        )
                    ost = op.tile([128, 512], dt.float32, tag="ost")
                    # copy on Act (DVE is the contended engine mid-kernel)
                    nc.scalar.copy(ost[:], po[:])
                    nc.sync.dma_start(
                        out[tt * 128:(tt + 1) * 128, n2 * 512:(n2 + 1) * 512],
                        ost[:],
                    )

            from collections import deque

            # Filler work items (qkv projections, v tiles, out projections)
            # streamed into the attention pipeline's PE gaps.  The attention
            # inner loop is Activation-paced (exp ~1040ns vs 4 matmuls
            # ~850ns per k-tile step), so the PE has a ~300-500ns deficit
            # per step that filler matmuls must cover; pacing them evenly
            # across ALL 160 steps (instead of draining greedily up front)
            # is what keeps the PE busy end to end.
            # Item = (deadline_block, kind, *args); FIFO is deadline-sorted.
            PE_MM = 213.0  # ns per 512-row bf16 matmul at full clock
            ITEM_COST = {"v": 8 * PE_MM, "proj": 9 * PE_MM, "out": 8 * PE_MM}
            DEFICIT_PER_J = 185.0   # Act busy excess over PE per k-tile step
            DIAG_EXTRA = 600.0      # exp->mask->pv latency on diagonal tiles
            BLOCK_BONUS = 600.0

            # proj items get a one-block-earlier deadline: the block's first
            # scores read qT/kT through the proj's DVE rope chain (~2.3us
            # after its last matmul), so draining at the consuming block
            # stalls the PE on DVE.  v items feed pv directly (short dep) and
            # can drain just in time.
            items = []
            for c4 in range(1, TC):  # head-pair 0's remaining chunks + v
                items.append((c4 - 1, "proj", 0, c4))
                items.append((c4 - 1, "proj", 4, c4))
                for tt in range(4 * c4, 4 * c4 + 4):
                    items.append((c4, "v", tt))
            for hp in range(1, 4):
                for c4 in range(TC):
                    items.append((hp * 4 + c4 - 1, "proj", hp, c4))
                    items.append((hp * 4 + c4 - 1, "proj", hp + 4, c4))
            items.sort(key=lambda it: it[0])
            fillers = deque(items)

            deficit = 0.0

            def run_item(it):
                if it[1] == "v":
                    v_tile(it[2])
                elif it[1] == "proj":
                    proj_rope(it[2], it[3])
                else:
                    out_proj(it[2])

            def pop_by_deficit():
                nonlocal deficit
                while fillers and deficit >= ITEM_COST[fillers[0][1]]:
                    it = fillers.popleft()
                    deficit -= ITEM_COST[it[1]]
                    run_item(it)

            # bootstrap: only block (0,0)'s dependencies
            proj_rope(0, 0, use_big=True)
            proj_rope(4, 0, use_big=True)
            for tt in range(4):
                v_tile(tt)

            for hp in range(4):
                for qc in range(TC):
                    bidx = hp * 4 + qc
                    # dependencies of this attention block must be emitted
                    while fillers and fillers[0][0] <= bidx:
                        run_item(fillers.popleft())
                    qsl = slice(qc * 512, (qc + 1) * 512)
                    jmax = 4 * qc + 3
                    pv0 = ppv.tile([65, 512], dt.float32, tag="pv")
                    pv1 = ppv.tile([65, 512], dt.float32, tag="pv")
                    for j in range(jmax + 1):
                        qs = max(0, j * 128 - qc * 512)
                        w = 512 - qs
                        diag = qs > 0 or j * 128 == qc * 512
                        # per-parity score tiles + exps: each parity's chain
                        # (score -> exp -> mask -> pv) releases its PSUM bank
                        # and unblocks its pv independently, halving the
                        # exp latency each pv waits on.
                        ex = ep.tile([128, 1024], dts, tag="ex")
                        ex_v = ex.rearrange("p (two q) -> p two q", two=2)
                        for par in range(2):
                            kb = par * 64
                            _lbl("score")
                            bigp = pb.tile([128, 512], dt.float32, tag="big")
                            nc.tensor.matmul(
                                bigp[:, qs:512],
                                kT_sb[kb:kb + 64, hp, j * 128:(j + 1) * 128],
                                qT_sb[kb:kb + 64, hp, qc * 512 + qs:(qc + 1) * 512],
                                start=True, stop=True,
                            )
                            _lbl("exp")
                            nc.scalar.activation(
                                ex_v[:, par, qs:512], bigp[:, qs:512],
                                mybir.ActivationFunctionType.Exp,
                                bias=0.0, scale=0.125,
                            )
                            _lbl("mask")
                            if diag:
                                # diagonal tile: zero the strictly-upper part
                                nc.vector.tensor_tensor(
                                    ex_v[:, par, qs:qs + 128],
                                    ex_v[:, par, qs:qs + 128],
                                    tri_sb[:, :],
                                    mybir.AluOpType.mult,
                                )
                        # fillers go BEFORE the pv matmuls: pv blocks on the
                        # exp (and mask) chain, and blocked instructions fill
                        # the PE's depth-4 wait queue — ready work emitted
                        # behind them could not issue.
                        deficit += DEFICIT_PER_J + (DIAG_EXTRA if diag else 0)
                        pop_by_deficit()
                        _lbl("pv")
                        for par in range(2):
                            h = 2 * hp + par
                            pv = pv0 if par == 0 else pv1
                            nc.tensor.matmul(
                                pv[:, qs:512],
                                v_sb[:, j, h * 65:(h + 1) * 65],
                                ex[:, par * 512 + qs:par * 512 + 512],
                                start=(j == 0), stop=(j == jmax),
                            )
                    _lbl("norm")
                    for par in range(2):
                        h = 2 * hp + par
                        kb = par * 64
                        pv = pv0 if par == 0 else pv1
                        rinv = np_.tile([1, 512], dt.float32, tag="rinv")
                        nc.vector.reciprocal(rinv[0:1, :], pv[64:65, :])
                        rb = np_.tile([64, 512], dt.float32, tag="rb")
                        nc.gpsimd.partition_broadcast(rb[:], rinv[0:1, :])
                        nc.vector.tensor_mul(
                            y_sb[kb:kb + 64, hp, qsl], pv[0:64, :], rb[:],
                        )
                    deficit += BLOCK_BONUS
                    if hp == 3:
                        # this chunk's y is final for all heads: stream out-proj
                        for tt in range(4 * qc, 4 * qc + 4):
                            fillers.append((16, "out", tt))
            while fillers:
                run_item(fillers.popleft())

    nc.compile()
    return nc


def _prep_core_inputs(x, W_qkv, b_qkv, W_out, g):
    """Host-side shard prep for head-group g (features g*512:(g+1)*512)."""
    fs = slice(g * 512, (g + 1) * 512)
    Wq = W_qkv[:, 0:1024][:, fs]          # [1024, 512]
    Wk = W_qkv[:, 1024:2048][:, fs]
    Wv_ = W_qkv[:, 2048:3072][:, fs]
    bq = b_qkv[0:1024][fs]
    bk = b_qkv[1024:2048][fs]
    bv_ = b_qkv[2048:3072][fs]

    Wqk_np = np.concatenate([Wq, Wk], axis=1)        # [1024, 1024]
    # [fs, p, ks, col]
    Wqk_np = Wqk_np.reshape(KSUB, 128, 8, 128).transpose(2, 1, 0, 3)
    Wv_np = Wv_.reshape(KSUB, 128, 512).transpose(1, 0, 2)
    Wo_np = W_out[fs, :].reshape(4, 128, 1024).transpose(1, 0, 2)
    bqk_np = np.concatenate([bq, bk]).reshape(8, 128).T.copy()   # [128, 8]
    bv_np = np.broadcast_to(bv_[None, :], (128, 512)).copy()

    return {
        "Wqk": np.ascontiguousarray(Wqk_np).astype(bf16),
        "Wv": np.ascontiguousarray(Wv_np).astype(bf16),
        "Wo": np.ascontiguousarray(Wo_np).astype(bf16),
        "bqk": np.ascontiguousarray(bqk_np).astype(np.float32),
        "bv": bv_np.astype(np.float32),
    }


def _shared_inputs():
    # rotation matrix: (R q)[d] = -q[d+32] for d<32, q[d-32] for 32<=d<64
    R64 = np.zeros((64, 64), dtype=np.float32)
    for d in range(32):
        R64[d, d + 32] = -1.0
        R64[d + 32, d] = 1.0
    R128 = np.zeros((128, 128), dtype=np.float32)
    R128[0:64, 0:64] = R64
    R128[64:128, 64:128] = R64
    RT_np = R128.T.copy()

    inv_freq = 1.0 / (10000.0 ** (np.arange(0, HD, 2, dtype=np.float32) / HD))
    t = np.arange(T, dtype=np.float32)
    freqs = np.outer(t, inv_freq)                     # [T, 32]
    p = np.arange(128)
    cos_np = np.cos(freqs[:, p % 32]).T.copy()        # [128, T]
    sin_np = np.sin(freqs[:, p % 32]).T.copy()

    tri_np = np.tril(np.ones((128, 128), dtype=np.float32)).T.copy()  # kk<=qq

    return {
        "RT": RT_np.astype(bf16),
        "cosd": np.ascontiguousarray(
            cos_np.reshape(128, TC, 512).transpose(1, 0, 2)).astype(bf16),
        "sind": np.ascontiguousarray(
            sin_np.reshape(128, TC, 512).transpose(1, 0, 2)).astype(bf16),
        "trid": np.ascontiguousarray(tri_np).astype(bf16),
    }


def run(x, W_qkv, b_qkv, W_out, b_out, trace=False):
    global _compiled
    if _compiled is None:
        _compiled = _build()
    nc = _compiled

    shared = _shared_inputs()
    group_inp = [_prep_core_inputs(x, W_qkv, b_qkv, W_out, g) for g in range(2)]

    in_maps = []
    for core in range(N_CORES):
        b, g = core // 2, core % 2
        # [c4, p, ks, q]
        xT_np = (x[b].reshape(TC, 512, KSUB, 128).transpose(0, 3, 2, 1))
        m = {"xT": np.ascontiguousarray(xT_np).astype(bf16)}
        m.update(group_inp[g])
        m.update(shared)
        in_maps.append(m)

    res = run_bass_kernel_spmd(
        nc, in_maps, core_ids=list(range(N_CORES)), trace=trace,
        stitch_traces=trace,
    )
    outp = np.empty((B, T, C), dtype=np.float32)
    for b in range(B):
        outp[b] = (res.results[2 * b]["out"] + res.results[2 * b + 1]["out"]
                   + b_out[None, :])
    return outp, res


def kernel(x, W_qkv, b_qkv, W_out, b_out):
    x = np.asarray(x, dtype=np.float32)
    W_qkv = np.asarray(W_qkv, dtype=np.float32)
    b_qkv = np.asarray(b_qkv, dtype=np.float32)
    W_out = np.asarray(W_out, dtype=np.float32)
    b_out = np.asarray(b_out, dtype=np.float32)
    outp, _ = run(x, W_qkv, b_qkv, W_out, b_out, trace=False)
    return outp

